# revision 1
# baseline (speedup 1.0000x reference)
"""LSTM (BaseRNN) Trainium2 kernel.

Problem: B=128, T=512, I=256, H=768 LSTM; returns (hiddenStates, cellStates)
each [B, T, H] fp32.

Strategy (data-parallel over batch, 8 cores x 16 rows):
  - Batch-major gate preactivations g_t = x_t W + h_{t-1} U accumulated in
    PSUM as [batch, gate_cols]; stationary operand = x^T / h^T chunks
    [128, 16] fp16, moving operand = W/U chunks [128, 384] fp16 (fp32r
    cannot be combined with tile_position, fp16 can; 1 cycle/row).
  - 4-way PE col-group tiling: gate q -> col group q (psum partitions
    32q..); every group's first matmul carries start=True (has_written
    clearing is per-partition-range).
  - Host permutes gate columns to (i, f, o, g~).  One ACT sigmoid with a
    per-partition scale vector (1 for i/f/o rows, 2 for g~ rows) covers all
    four gates; tanh(x) = 2*sigmoid(2x)-1 is reconstructed by a cheap DVE
    tensor_scalar.
  - The step is processed in two independent column halves (psum banks) so
    consecutive ops pipeline across engines; gate values, c, and h are fp16
    (DVE 2x mode) which keeps the serial dependency chain short.
  - h_t half is re-transposed via 3 PE-transposes + 1 ACT copy into the
    fp16 h^T stationary for the next step.
  - hs/cs stream to DRAM as fp16; the host upcasts to fp32.
"""

import numpy as np

import concourse.bass as bass
import concourse.bacc as bacc
import concourse.tile as tile
from concourse import mybir
from concourse.bass_utils import run_bass_kernel_spmd
from concourse.masks import make_identity

B, T, I, H = 128, 512, 256, 768
NCORES = 8
NB = B // NCORES  # 16
KX = I // 128  # 2 x chunks
KH = H // 128  # 6 h chunks
NK = KX + KH  # 8 contraction waves
NHALF = H // 2  # 384: per-gate psum half (one matmul's N)
F32 = mybir.dt.float32
F16 = mybir.dt.float16
XBLK = 32  # x-stream block (steps per DMA)

MM_DT = F16  # matmul operand dtype


def build_lstm(nb=NB, t_steps=T, has_b=False):
    nc = bacc.Bacc(None, target_bir_lowering=False)

    xT_d = nc.dram_tensor("xT", [t_steps, KX, 128, nb], F32, kind="ExternalInput")
    h0_d = nc.dram_tensor("h0", [nb, H], F32, kind="ExternalInput")
    c0_d = nc.dram_tensor("c0", [nb, 2, NHALF], F16, kind="ExternalInput")
    w_d = nc.dram_tensor("w", [KX, 128, 4 * H], F32, kind="ExternalInput")
    u_d = nc.dram_tensor("u", [KH, 128, 4 * H], F32, kind="ExternalInput")
    b_d = nc.dram_tensor("b", [1, 4 * H], F32, kind="ExternalInput")
    hs_d = nc.dram_tensor("hs", [t_steps, 128, KH * NB], F16, kind="ExternalOutput")
    cs_d = nc.dram_tensor("cs", [nb, t_steps, 2, NHALF], F16, kind="ExternalOutput")

    SIG = mybir.ActivationFunctionType.Sigmoid
    TANH = mybir.ActivationFunctionType.Tanh
    MULT = mybir.AluOpType.mult
    ADD = mybir.AluOpType.add

    with tile.TileContext(nc) as tc:
        with (
            tc.tile_pool(name="consts", bufs=1) as consts,
            tc.tile_pool(name="xs", bufs=2) as xs_pool,
            tc.tile_pool(name="gsb", bufs=3) as gsb,
            tc.tile_pool(name="ew", bufs=3) as ew,
            tc.tile_pool(name="state", bufs=2) as state,
            tc.tile_pool(name="pg", bufs=2, space="PSUM") as pg,
            tc.tile_pool(name="pt", bufs=2, space="PSUM") as pt,
        ):
            ident16 = consts.tile([nb, nb], F16)
            make_identity(nc, ident16)
            idento = consts.tile([64 + nb, nb], F16)
            make_identity(nc, idento[64 : 64 + nb])
            # per-partition activation input scale: 1 for i/f/o rows,
            # 2 for g~ rows (tanh(x) = 2*sigmoid(2x) - 1)
            sc = consts.tile([112, 1], F32)
            nc.vector.memset(sc[0:96], 1.0)
            nc.vector.memset(sc[96:112], 2.0)

            # weights: DMA f32 staging -> round-convert to fp16
            w_sb = consts.tile([128, KX, 4 * H], MM_DT)
            u_sb = consts.tile([128, KH, 4 * H], MM_DT)
            for k in range(KX):
                stg = xs_pool.tile([128, 4 * H], F32, tag="WSTG")
                nc.sync.dma_start(out=stg, in_=w_d[k, :, :])
                nc.vector.tensor_copy(w_sb[:, k, :], stg)
            for k in range(KH):
                stg = xs_pool.tile([128, 4 * H], F32, tag="WSTG")
                nc.sync.dma_start(out=stg, in_=u_d[k, :, :])
                nc.vector.tensor_copy(u_sb[:, k, :], stg)
            if has_b:
                b_sb = consts.tile([1, 4 * H], F32)
                nc.sync.dma_start(out=b_sb, in_=b_d[:, :])

            # ---- initial state ----
            h0_sb = consts.tile([nb, H], F32)
            nc.sync.dma_start(out=h0_sb, in_=h0_d[:, :])
            c_prev = [None, None]
            for n2 in range(2):
                ct = state.tile([32 + nb, NHALF], F16, tag=f"C{n2}")
                nc.sync.dma_start(out=ct[32 : 32 + nb], in_=c0_d[:, n2, :])
                c_prev[n2] = ct

            h0_16 = consts.tile([nb, H], F16)
            nc.vector.tensor_copy(h0_16, h0_sb)
            ht_prev = state.tile([128, KH * nb], MM_DT, tag="HT")
            for n2 in range(2):
                ht0_ps = pt.tile([128, 3 * nb], F16, tag="tcTps")
                for j in range(3):
                    ck = 3 * n2 + j
                    nc.tensor.transpose(
                        ht0_ps[:, j * nb : (j + 1) * nb],
                        h0_16[:, ck * 128 : (ck + 1) * 128],
                        ident16,
                    )
                nc.scalar.copy(
                    out=ht_prev[:, 3 * n2 * nb : (3 * n2 + 3) * nb], in_=ht0_ps
                )

            x_tile = None
            for t in range(t_steps):
                trel = t % XBLK
                if trel == 0:
                    xstg = xs_pool.tile([128, XBLK, KX, nb], F32, tag="XSTG")
                    nblk = min(XBLK, t_steps - t)
                    nc.sync.dma_start(
                        out=xstg[:, 0:nblk],
                        in_=xT_d[t : t + nblk].rearrange("t k p b -> p t k b"),
                    )
                    x_tile = xs_pool.tile([128, XBLK, KX, nb], MM_DT, tag="X")
                    nc.vector.tensor_copy(x_tile[:, 0:nblk], xstg[:, 0:nblk])

                gates = pg.tile([128, 2, 512], F32, tag="gates")
                ht_new = state.tile([128, KH * nb], MM_DT, tag="HT")
                for n2 in range(2):
                    for k in range(NK):
                        for q in range(4):
                            col = q * H + n2 * NHALF
                            if k < KX:
                                stat = x_tile[:, trel, k, :]
                                mov = w_sb[:, k, col : col + NHALF]
                            else:
                                stat = ht_prev[:, (k - KX) * nb : (k - KX + 1) * nb]
                                mov = u_sb[:, k - KX, col : col + NHALF]
                            nc.tensor.matmul(
                                gates[32 * q : 32 * q + nb, n2, 0:NHALF],
                                stat,
                                mov,
                                start=(k == 0),
                                stop=(k == NK - 1),
                                tile_position=(0, 32 * q),
                                skip_group_check=True,
                            )

                    if has_b:
                        for q, base in ((0, 0), (1, 32), (2, 64), (3, 96)):
                            bq = b_sb[:, q * H + n2 * NHALF : q * H + (n2 + 1) * NHALF]
                            bq = bass.AP(
                                tensor=bq.tensor, offset=bq.offset,
                                ap=[[0, nb]] + bq.ap[1:],
                            )
                            nc.vector.tensor_add(
                                gates[base : base + nb, n2, 0:NHALF],
                                gates[base : base + nb, n2, 0:NHALF],
                                bq,
                            )

                # --- elementwise, halves interleaved stage-by-stage so the
                # two banks' chains overlap (engines run in program order) ---
                # S2 rows: i'@0:16, f'@32:48, o'@64:80, sigmoid(2g)@96:112
                S2, C, G, T1, TC = {}, {}, {}, {}, {}
                for n2 in range(2):
                    S2[n2] = gsb.tile([112, NHALF], F16, tag=f"S2{n2}", name=f"S2_{n2}")
                    nc.scalar.activation(
                        out=S2[n2], in_=gates[0:112, n2, 0:NHALF],
                        func=SIG, scale=sc,
                    )
                for n2 in range(2):
                    # f' * c  (into C rows 32:48)
                    C[n2] = state.tile([32 + nb, NHALF], F16, tag=f"C{n2}", name=f"C_{n2}")
                    nc.vector.tensor_mul(
                        C[n2][32 : 32 + nb], S2[n2][32 : 32 + nb],
                        c_prev[n2][32 : 32 + nb],
                    )
                for n2 in range(2):
                    # g~ = 2*sigmoid(2g) - 1
                    G[n2] = gsb.tile([nb, NHALF], F16, tag=f"G{n2}", name=f"G_{n2}")
                    nc.vector.tensor_scalar(
                        G[n2], S2[n2][96:112], 2.0, -1.0, MULT, ADD
                    )
                for n2 in range(2):
                    # i' * g~ (out-shift to rows 32:48)
                    T1[n2] = ew.tile([32 + nb, NHALF], F16, tag=f"T1{n2}", name=f"T1_{n2}")
                    nc.vector.tensor_mul(
                        T1[n2][32 : 32 + nb], S2[n2][0:nb], G[n2]
                    )
                for n2 in range(2):
                    nc.vector.tensor_add(
                        C[n2][32 : 32 + nb], C[n2][32 : 32 + nb],
                        T1[n2][32 : 32 + nb],
                    )
                for n2 in range(2):
                    TC[n2] = ew.tile([64 + nb, NHALF], F16, tag=f"TC{n2}", name=f"TC_{n2}")
                    nc.scalar.activation(
                        out=TC[n2][64 : 64 + nb], in_=C[n2][32 : 32 + nb],
                        func=TANH,
                    )
                # o'^T: transpose o' into psum then copy to SBUF (off the
                # critical chain - o' is ready right after the sigmoid)
                oT = {}
                for n2 in range(2):
                    oT_ps = pt.tile([128, 3 * nb], F16, tag="oTps", name=f"oTps_{n2}")
                    for j in range(3):
                        nc.tensor.transpose(
                            oT_ps[:, j * nb : (j + 1) * nb],
                            S2[n2][64 : 64 + nb, j * 128 : (j + 1) * 128],
                            idento[64 : 64 + nb],
                        )
                    oT[n2] = ew.tile([128, 3 * nb], F16, tag=f"oT{n2}", name=f"oT_{n2}")
                    nc.scalar.copy(out=oT[n2], in_=oT_ps)
                # tc^T via PE transpose (stays in psum), then
                # h^T = o'^T * tc^T written directly into the stationary tile
                for n2 in range(2):
                    tcT_ps = pt.tile([128, 3 * nb], F16, tag="tcTps", name=f"tcTps_{n2}")
                    for j in range(3):
                        nc.tensor.transpose(
                            tcT_ps[:, j * nb : (j + 1) * nb],
                            TC[n2][64 : 64 + nb, j * 128 : (j + 1) * 128],
                            idento[64 : 64 + nb],
                        )
                    nc.vector.tensor_mul(
                        ht_new[:, 3 * n2 * nb : (3 * n2 + 3) * nb],
                        oT[n2], tcT_ps,
                    )
                for n2 in range(2):
                    nc.sync.dma_start(
                        out=cs_d[:, t, n2], in_=C[n2][32 : 32 + nb]
                    )
                    c_prev[n2] = C[n2]
                nc.sync.dma_start(out=hs_d[t, :, :], in_=ht_new)

                ht_prev = ht_new

    nc.finalize()
    return nc


# Column permutation: reference gate order (i, f, g~, o) -> kernel (i, f, o, g~)
def _gate_perm():
    return np.concatenate(
        [np.arange(0, H), np.arange(H, 2 * H), np.arange(3 * H, 4 * H),
         np.arange(2 * H, 3 * H)]
    )


def _prep_core_inputs(input_, h0, c0, Wp, Up, bp, t_steps):
    nb = input_.shape[0]
    xT = np.ascontiguousarray(
        input_[:, :t_steps].transpose(1, 2, 0).reshape(t_steps, KX, 128, nb)
    )
    return {
        "xT": xT,
        "h0": np.ascontiguousarray(h0),
        "c0": np.ascontiguousarray(c0.reshape(nb, 2, NHALF).astype(np.float16)),
        "w": Wp,
        "u": Up,
        "b": bp,
    }


def run(input, hiddenState, cellState, W, U, b, t_steps=T, trace=False):
    input = np.asarray(input, np.float32)
    hiddenState = np.asarray(hiddenState, np.float32)
    cellState = np.asarray(cellState, np.float32)
    W = np.asarray(W, np.float32)
    U = np.asarray(U, np.float32)
    b = np.asarray(b, np.float32)

    perm = _gate_perm()
    Wp = np.ascontiguousarray(W[:, perm].reshape(KX, 128, 4 * H))
    Up = np.ascontiguousarray(U[:, perm].reshape(KH, 128, 4 * H))
    bp = np.ascontiguousarray(b[perm].reshape(1, 4 * H))
    has_b = bool(np.any(b))

    nc = build_lstm(NB, t_steps, has_b)
    in_maps = []
    for c in range(NCORES):
        bs = slice(c * NB, (c + 1) * NB)
        in_maps.append(
            _prep_core_inputs(
                input[bs], hiddenState[bs], cellState[bs], Wp, Up, bp, t_steps
            )
        )
    res = run_bass_kernel_spmd(
        nc, in_maps, core_ids=list(range(NCORES)), trace=trace
    )

    hs = np.empty((B, t_steps, H), np.float32)
    cs = np.empty((B, t_steps, H), np.float32)
    for c in range(NCORES):
        bs = slice(c * NB, (c + 1) * NB)
        ht = res.results[c]["hs"].astype(np.float32)  # [t, 128, 6*16]
        ht = ht.reshape(t_steps, 128, KH, NB)
        hs[bs] = ht.transpose(3, 0, 2, 1).reshape(NB, t_steps, H)
        cs[bs] = res.results[c]["cs"].astype(np.float32).reshape(NB, t_steps, H)
    return (hs, cs), res


def kernel(input, hiddenState, cellState, W, U, b):
    (hs, cs), _ = run(input, hiddenState, cellState, W, U, b)
    return hs, cs



# revision 5
# speedup vs baseline: 1.0540x; 1.0540x over previous
"""LSTM (BaseRNN) Trainium2 kernel.

Problem: B=128, T=512, I=256, H=768 LSTM; returns (hiddenStates, cellStates)
each [B, T, H] fp32.

Strategy (data-parallel over batch, 8 cores x 16 rows):
  - Batch-major gate preactivations g_t = x_t W + h_{t-1} U accumulated in
    PSUM as [batch, gate_cols]; stationary operand = x^T / h^T chunks
    [128, 16] fp16, moving operand = W/U chunks [128, 384] fp16.
  - 4-way PE col-group tiling: gate q -> col group q (psum partitions
    32q..); every group's first matmul carries start=True.
  - Host permutes gate columns to (i, f, o, g~).  One ACT sigmoid with a
    per-partition scale vector (1 for i/f/o rows, 2 for g~ rows) covers all
    four gates; tanh(x) = 2*sigmoid(2x)-1 is reconstructed by a cheap DVE
    tensor_scalar.
  - The step is processed in two independent column halves (psum banks);
    gate values, c, and h are fp16.
  - Pipelining for latency (the recurrent cycle is the bound):
      * next step's x-waves are emitted right after this step's h-waves so
        the PE stays busy (HAM stays warm) during the elementwise phase;
      * h-waves are ordered [h0-chunks psum0, h0-chunks psum1, h1-chunks
        psum0, h1-chunks psum1] so each half's sigmoid fires as early as
        possible;
      * G is written into spare partitions of the c_prev tile so f'*c and
        i'*g~ fuse into one [48,384] DVE multiply;
      * oT psum->sbuf copies run on DVE (ACT does only sigmoid+tanh).
  - h_t half is re-transposed via 3 PE-transposes into the fp16 h^T
    stationary for the next step.
  - hs/cs stream to DRAM as fp16; the host upcasts to fp32.
"""

import numpy as np

import concourse.bass as bass
import concourse.bacc as bacc
import concourse.tile as tile
from concourse import mybir
from concourse.bass_utils import run_bass_kernel_spmd
from concourse.masks import make_identity

B, T, I, H = 128, 512, 256, 768
NCORES = 8
NB = B // NCORES  # 16
KX = I // 128  # 2 x chunks
KH = H // 128  # 6 h chunks
NK = KX + KH  # 8 contraction waves
NHALF = H // 2  # 384: per-gate psum half (one matmul's N)
F32 = mybir.dt.float32
F16 = mybir.dt.float16
XBLK = 32  # x-stream block (steps per DMA)

MM_DT = F16  # matmul operand dtype


def build_lstm(nb=NB, t_steps=T, has_b=False):
    nc = bacc.Bacc(None, target_bir_lowering=False)

    xT_d = nc.dram_tensor("xT", [t_steps, KX, 128, nb], F32, kind="ExternalInput")
    h0_d = nc.dram_tensor("h0", [nb, H], F32, kind="ExternalInput")
    c0_d = nc.dram_tensor("c0", [nb, 2, NHALF], F16, kind="ExternalInput")
    w_d = nc.dram_tensor("w", [KX, 128, 4 * H], F32, kind="ExternalInput")
    u_d = nc.dram_tensor("u", [KH, 128, 4 * H], F32, kind="ExternalInput")
    b_d = nc.dram_tensor("b", [1, 4 * H], F32, kind="ExternalInput")
    hs_d = nc.dram_tensor("hs", [t_steps, 128, KH * NB], F16, kind="ExternalOutput")
    cs_d = nc.dram_tensor("cs", [nb, t_steps, 2, NHALF], F16, kind="ExternalOutput")

    SIG = mybir.ActivationFunctionType.Sigmoid
    TANH = mybir.ActivationFunctionType.Tanh
    MULT = mybir.AluOpType.mult
    ADD = mybir.AluOpType.add

    with tile.TileContext(nc) as tc:
        with (
            tc.tile_pool(name="consts", bufs=1) as consts,
            tc.tile_pool(name="xs", bufs=2) as xs_pool,
            tc.tile_pool(name="gsb", bufs=3) as gsb,
            tc.tile_pool(name="ew", bufs=3) as ew,
            tc.tile_pool(name="state", bufs=2) as state,
            tc.tile_pool(name="pg", bufs=2, space="PSUM") as pg,
            tc.tile_pool(name="pt", bufs=2, space="PSUM") as pt,
        ):
            ident16 = consts.tile([nb, nb], F16)
            make_identity(nc, ident16)
            idento = consts.tile([64 + nb, nb], F16)
            make_identity(nc, idento[64 : 64 + nb])
            # per-partition activation input scale: 1 for i/f/o rows,
            # 2 for g~ rows (tanh(x) = 2*sigmoid(2x) - 1)
            sc = consts.tile([112, 1], F32)
            nc.vector.memset(sc[0:96], 1.0)
            nc.vector.memset(sc[96:112], 2.0)

            # weights: DMA f32 staging -> round-convert to fp16
            w_sb = consts.tile([128, KX, 4 * H], MM_DT)
            u_sb = consts.tile([128, KH, 4 * H], MM_DT)
            for k in range(KX):
                stg = xs_pool.tile([128, 4 * H], F32, tag="WSTG")
                nc.sync.dma_start(out=stg, in_=w_d[k, :, :])
                nc.vector.tensor_copy(w_sb[:, k, :], stg)
            for k in range(KH):
                stg = xs_pool.tile([128, 4 * H], F32, tag="WSTG")
                nc.sync.dma_start(out=stg, in_=u_d[k, :, :])
                nc.vector.tensor_copy(u_sb[:, k, :], stg)
            if has_b:
                b_sb = consts.tile([1, 4 * H], F32)
                nc.sync.dma_start(out=b_sb, in_=b_d[:, :])

            # ---- initial state ----
            h0_sb = consts.tile([nb, H], F32)
            nc.sync.dma_start(out=h0_sb, in_=h0_d[:, :])
            c_prev = [None, None]
            for n2 in range(2):
                ct = state.tile([32 + nb, NHALF], F16, tag=f"C{n2}")
                nc.sync.dma_start(out=ct[32 : 32 + nb], in_=c0_d[:, n2, :])
                c_prev[n2] = ct

            h0_16 = consts.tile([nb, H], F16)
            nc.vector.tensor_copy(h0_16, h0_sb)
            ht_prev = state.tile([128, KH * nb], MM_DT, tag="HT")
            for n2 in range(2):
                ht0_ps = pt.tile([128, 3 * nb], F16, tag="tcTps")
                for j in range(3):
                    ck = 3 * n2 + j
                    nc.tensor.transpose(
                        ht0_ps[:, j * nb : (j + 1) * nb],
                        h0_16[:, ck * 128 : (ck + 1) * 128],
                        ident16,
                    )
                nc.scalar.copy(
                    out=ht_prev[:, 3 * n2 * nb : (3 * n2 + 3) * nb], in_=ht0_ps
                )

            def load_xblk(t0):
                """DMA + f16-convert the x block starting at step t0."""
                xstg = xs_pool.tile([128, XBLK, KX, nb], F32, tag="XSTG")
                nblk = min(XBLK, t_steps - t0)
                nc.sync.dma_start(
                    out=xstg[:, 0:nblk],
                    in_=xT_d[t0 : t0 + nblk].rearrange("t k p b -> p t k b"),
                )
                xt = xs_pool.tile([128, XBLK, KX, nb], MM_DT, tag="X")
                # convert on the otherwise-idle GpSimd engine (SBUF->SBUF)
                nc.gpsimd.tensor_copy(xt[:, 0:nblk], xstg[:, 0:nblk])
                return xt

            def x_waves(gates_tile, xt, trel, add_bias):
                """Emit the x-projection waves (accumulation starters)."""
                for n2 in range(2):
                    for k in range(KX):
                        for q in range(4):
                            col = q * H + n2 * NHALF
                            nc.tensor.matmul(
                                gates_tile[32 * q : 32 * q + nb, n2, 0:NHALF],
                                xt[:, trel, k, :],
                                w_sb[:, k, col : col + NHALF],
                                start=(k == 0),
                                stop=False,
                                tile_position=(0, 32 * q),
                                skip_group_check=True,
                            )
                if add_bias:
                    for n2 in range(2):
                        for q, base in ((0, 0), (1, 32), (2, 64), (3, 96)):
                            bq = b_sb[:, q * H + n2 * NHALF : q * H + (n2 + 1) * NHALF]
                            bq = bass.AP(
                                tensor=bq.tensor, offset=bq.offset,
                                ap=[[0, nb]] + bq.ap[1:],
                            )
                            nc.vector.tensor_add(
                                gates_tile[base : base + nb, n2, 0:NHALF],
                                gates_tile[base : base + nb, n2, 0:NHALF],
                                bq,
                            )

            # x block 0 + step-0 x-waves
            x_tile = load_xblk(0)
            gates_cur = pg.tile([128, 2, 512], F32, tag="gates")
            # bias path handled via DVE adds after accumulation finishes, so
            # hoisted x-waves stay start-only; adds are emitted with h-waves.
            x_waves(gates_cur, x_tile, 0, False)

            for t in range(t_steps):
                trel1 = (t + 1) % XBLK
                x_next = x_tile
                if t + 1 < t_steps and trel1 == 0:
                    x_next = load_xblk(t + 1)

                # --- h-waves for step t: h0-dependent chunks (0-2) for both
                # psum halves first, then h1-dependent chunks (3-5) ---
                for ckgrp in range(2):
                    for n2 in range(2):
                        for kc in range(3):
                            ck = 3 * ckgrp + kc
                            for q in range(4):
                                col = q * H + n2 * NHALF
                                nc.tensor.matmul(
                                    gates_cur[32 * q : 32 * q + nb, n2, 0:NHALF],
                                    ht_prev[:, ck * nb : (ck + 1) * nb],
                                    u_sb[:, ck, col : col + NHALF],
                                    start=False,
                                    stop=(ck == KH - 1),
                                    tile_position=(0, 32 * q),
                                    skip_group_check=True,
                                )

                if has_b:
                    for n2 in range(2):
                        for q, base in ((0, 0), (1, 32), (2, 64), (3, 96)):
                            bq = b_sb[:, q * H + n2 * NHALF : q * H + (n2 + 1) * NHALF]
                            bq = bass.AP(
                                tensor=bq.tensor, offset=bq.offset,
                                ap=[[0, nb]] + bq.ap[1:],
                            )
                            nc.vector.tensor_add(
                                gates_cur[base : base + nb, n2, 0:NHALF],
                                gates_cur[base : base + nb, n2, 0:NHALF],
                                bq,
                            )

                # --- next step's x-waves: keep the PE busy during the
                # elementwise phase (HAM stays warm) ---
                gates_next = None
                if t + 1 < t_steps:
                    gates_next = pg.tile([128, 2, 512], F32, tag="gates")
                    x_waves(gates_next, x_next, trel1, False)

                # --- elementwise, halves interleaved stage-by-stage ---
                # S2 rows: i'@0:16, f'@32:48, o'@64:80, sigmoid(2g)@96:112
                S2, C, G, T1, TC = {}, {}, {}, {}, {}
                for n2 in range(2):
                    S2[n2] = gsb.tile([112, NHALF], F16, tag=f"S2{n2}", name=f"S2_{n2}")
                    nc.scalar.activation(
                        out=S2[n2], in_=gates_cur[0:112, n2, 0:NHALF],
                        func=SIG, scale=sc,
                    )
                for n2 in range(2):
                    # g~ = 2*sigmoid(2g) - 1
                    G[n2] = gsb.tile([nb, NHALF], F16, tag=f"G{n2}", name=f"G_{n2}")
                    nc.vector.tensor_scalar(
                        G[n2], S2[n2][96:112], 2.0, -1.0, MULT, ADD
                    )
                for n2 in range(2):
                    # f' * c  (into C rows 32:48)
                    C[n2] = state.tile([32 + nb, NHALF], F16, tag=f"C{n2}", name=f"C_{n2}")
                    nc.vector.tensor_mul(
                        C[n2][32 : 32 + nb], S2[n2][32 : 32 + nb],
                        c_prev[n2][32 : 32 + nb],
                    )
                for n2 in range(2):
                    # i' * g~ (out-shift to rows 32:48)
                    T1[n2] = ew.tile([32 + nb, NHALF], F16, tag=f"T1{n2}", name=f"T1_{n2}")
                    nc.vector.tensor_mul(
                        T1[n2][32 : 32 + nb], S2[n2][0:nb], G[n2]
                    )
                for n2 in range(2):
                    nc.vector.tensor_add(
                        C[n2][32 : 32 + nb], C[n2][32 : 32 + nb],
                        T1[n2][32 : 32 + nb],
                    )
                # o'^T: transpose o' into psum then DVE-copy to SBUF (off the
                # critical chain - o' is ready right after the sigmoid)
                oT_ps, oT = {}, {}
                for n2 in range(2):
                    oT_ps[n2] = pt.tile([128, 3 * nb], F16, tag="oTps", name=f"oTps_{n2}")
                    for j in range(3):
                        nc.tensor.transpose(
                            oT_ps[n2][:, j * nb : (j + 1) * nb],
                            S2[n2][64 : 64 + nb, j * 128 : (j + 1) * 128],
                            idento[64 : 64 + nb],
                        )
                for n2 in range(2):
                    TC[n2] = ew.tile([64 + nb, NHALF], F16, tag=f"TC{n2}", name=f"TC_{n2}")
                    nc.scalar.activation(
                        out=TC[n2][64 : 64 + nb], in_=C[n2][32 : 32 + nb],
                        func=TANH,
                    )
                for n2 in range(2):
                    oT[n2] = ew.tile([128, 3 * nb], F16, tag=f"oT{n2}", name=f"oT_{n2}")
                    nc.vector.tensor_copy(oT[n2], oT_ps[n2])
                # tc^T via PE transpose (stays in psum), then
                # h^T = o'^T * tc^T written directly into the stationary tile
                ht_new = state.tile([128, KH * nb], MM_DT, tag="HT")
                for n2 in range(2):
                    tcT_ps = pt.tile([128, 3 * nb], F16, tag="tcTps", name=f"tcTps_{n2}")
                    for j in range(3):
                        nc.tensor.transpose(
                            tcT_ps[:, j * nb : (j + 1) * nb],
                            TC[n2][64 : 64 + nb, j * 128 : (j + 1) * 128],
                            idento[64 : 64 + nb],
                        )
                    nc.vector.tensor_mul(
                        ht_new[:, 3 * n2 * nb : (3 * n2 + 3) * nb],
                        oT[n2], tcT_ps,
                    )
                for n2 in range(2):
                    nc.sync.dma_start(
                        out=cs_d[:, t, n2], in_=C[n2][32 : 32 + nb]
                    )
                    c_prev[n2] = C[n2]
                nc.sync.dma_start(out=hs_d[t, :, :], in_=ht_new)

                ht_prev = ht_new
                gates_cur = gates_next
                x_tile = x_next

    nc.finalize()
    return nc


# Column permutation: reference gate order (i, f, g~, o) -> kernel (i, f, o, g~)
def _gate_perm():
    return np.concatenate(
        [np.arange(0, H), np.arange(H, 2 * H), np.arange(3 * H, 4 * H),
         np.arange(2 * H, 3 * H)]
    )


def _prep_core_inputs(input_, h0, c0, Wp, Up, bp, t_steps):
    nb = input_.shape[0]
    xT = np.ascontiguousarray(
        input_[:, :t_steps].transpose(1, 2, 0).reshape(t_steps, KX, 128, nb)
    )
    return {
        "xT": xT,
        "h0": np.ascontiguousarray(h0),
        "c0": np.ascontiguousarray(c0.reshape(nb, 2, NHALF).astype(np.float16)),
        "w": Wp,
        "u": Up,
        "b": bp,
    }


def run(input, hiddenState, cellState, W, U, b, t_steps=T, trace=False):
    input = np.asarray(input, np.float32)
    hiddenState = np.asarray(hiddenState, np.float32)
    cellState = np.asarray(cellState, np.float32)
    W = np.asarray(W, np.float32)
    U = np.asarray(U, np.float32)
    b = np.asarray(b, np.float32)

    perm = _gate_perm()
    Wp = np.ascontiguousarray(W[:, perm].reshape(KX, 128, 4 * H))
    Up = np.ascontiguousarray(U[:, perm].reshape(KH, 128, 4 * H))
    bp = np.ascontiguousarray(b[perm].reshape(1, 4 * H))
    has_b = bool(np.any(b))

    nc = build_lstm(NB, t_steps, has_b)
    in_maps = []
    for c in range(NCORES):
        bs = slice(c * NB, (c + 1) * NB)
        in_maps.append(
            _prep_core_inputs(
                input[bs], hiddenState[bs], cellState[bs], Wp, Up, bp, t_steps
            )
        )
    res = run_bass_kernel_spmd(
        nc, in_maps, core_ids=list(range(NCORES)), trace=trace
    )

    hs = np.empty((B, t_steps, H), np.float32)
    cs = np.empty((B, t_steps, H), np.float32)
    for c in range(NCORES):
        bs = slice(c * NB, (c + 1) * NB)
        ht = res.results[c]["hs"].astype(np.float32)  # [t, 128, 6*16]
        ht = ht.reshape(t_steps, 128, KH, NB)
        hs[bs] = ht.transpose(3, 0, 2, 1).reshape(NB, t_steps, H)
        cs[bs] = res.results[c]["cs"].astype(np.float32).reshape(NB, t_steps, H)
    return (hs, cs), res


def kernel(input, hiddenState, cellState, W, U, b):
    (hs, cs), _ = run(input, hiddenState, cellState, W, U, b)
    return hs, cs


# revision 6
# speedup vs baseline: 1.0661x; 1.0115x over previous
"""LSTM (BaseRNN) Trainium2 kernel.

Problem: B=128, T=512, I=256, H=768 LSTM; returns (hiddenStates, cellStates)
each [B, T, H] fp32.

Strategy (data-parallel over batch, 8 cores x 16 rows):
  - Batch-major gate preactivations g_t = x_t W + h_{t-1} U accumulated in
    PSUM as [batch, gate_cols]; stationary operand = x^T / h^T chunks
    [128, 16] fp16, moving operand = W/U chunks [128, 384] fp16.
  - 4-way PE col-group tiling: gate q -> col group q (psum partitions
    32q..); every group's first matmul carries start=True.
  - Host permutes gate columns to (i, f, o, g~).  One ACT sigmoid with a
    per-partition scale vector (1 for i/f/o rows, 2 for g~ rows) covers all
    four gates; tanh(x) = 2*sigmoid(2x)-1 is reconstructed by a cheap DVE
    tensor_scalar.
  - The step is processed in two independent column halves (psum banks);
    gate values, c, and h are fp16.
  - Pipelining for latency (the recurrent cycle is the bound):
      * next step's x-waves are emitted right after this step's h-waves so
        the PE stays busy (HAM stays warm) during the elementwise phase;
      * h-waves are ordered [h0-chunks psum0, h0-chunks psum1, h1-chunks
        psum0, h1-chunks psum1] so each half's sigmoid fires as early as
        possible;
      * G is written into spare partitions of the c_prev tile so f'*c and
        i'*g~ fuse into one [48,384] DVE multiply;
      * oT psum->sbuf copies run on DVE (ACT does only sigmoid+tanh).
  - h_t half is re-transposed via 3 PE-transposes into the fp16 h^T
    stationary for the next step.
  - hs/cs stream to DRAM as fp16; the host upcasts to fp32.
"""

import numpy as np

import concourse.bass as bass
import concourse.bacc as bacc
import concourse.tile as tile
from concourse import mybir
from concourse.bass_utils import run_bass_kernel_spmd
from concourse.masks import make_identity

B, T, I, H = 128, 512, 256, 768
NCORES = 8
NB = B // NCORES  # 16
KX = I // 128  # 2 x chunks
KH = H // 128  # 6 h chunks
NK = KX + KH  # 8 contraction waves
NHALF = H // 2  # 384: per-gate psum half (one matmul's N)
F32 = mybir.dt.float32
F16 = mybir.dt.float16
XBLK = 32  # x-stream block (steps per DMA)

MM_DT = F16  # matmul operand dtype


def build_lstm(nb=NB, t_steps=T, has_b=False):
    nc = bacc.Bacc(None, target_bir_lowering=False)

    xT_d = nc.dram_tensor("xT", [t_steps, KX, 128, nb], F32, kind="ExternalInput")
    h0_d = nc.dram_tensor("h0", [nb, H], F32, kind="ExternalInput")
    c0_d = nc.dram_tensor("c0", [nb, 2, NHALF], F16, kind="ExternalInput")
    w_d = nc.dram_tensor("w", [KX, 128, 4 * H], F32, kind="ExternalInput")
    u_d = nc.dram_tensor("u", [KH, 128, 4 * H], F32, kind="ExternalInput")
    b_d = nc.dram_tensor("b", [1, 4 * H], F32, kind="ExternalInput")
    hs_d = nc.dram_tensor("hs", [t_steps, 128, KH * NB], F16, kind="ExternalOutput")
    cs_d = nc.dram_tensor("cs", [nb, t_steps, 2, NHALF], F16, kind="ExternalOutput")

    SIG = mybir.ActivationFunctionType.Sigmoid
    TANH = mybir.ActivationFunctionType.Tanh
    MULT = mybir.AluOpType.mult
    ADD = mybir.AluOpType.add

    with tile.TileContext(nc) as tc:
        with (
            tc.tile_pool(name="consts", bufs=1) as consts,
            tc.tile_pool(name="xs", bufs=2) as xs_pool,
            tc.tile_pool(name="gsb", bufs=3) as gsb,
            tc.tile_pool(name="ew", bufs=3) as ew,
            tc.tile_pool(name="state", bufs=2) as state,
            tc.tile_pool(name="pg", bufs=2, space="PSUM") as pg,
            tc.tile_pool(name="pt", bufs=2, space="PSUM") as pt,
        ):
            ident16 = consts.tile([nb, nb], F16)
            make_identity(nc, ident16)
            idento = consts.tile([64 + nb, nb], F16)
            make_identity(nc, idento[64 : 64 + nb])
            # per-partition activation input scale: 1 for i/f/o rows,
            # 2 for g~ rows (tanh(x) = 2*sigmoid(2x) - 1)
            sc = consts.tile([112, 1], F32)
            nc.vector.memset(sc[0:96], 1.0)
            nc.vector.memset(sc[96:112], 2.0)

            # weights: DMA f32 staging -> round-convert to fp16
            w_sb = consts.tile([128, KX, 4 * H], MM_DT)
            u_sb = consts.tile([128, KH, 4 * H], MM_DT)
            for k in range(KX):
                stg = xs_pool.tile([128, 4 * H], F32, tag="WSTG")
                nc.sync.dma_start(out=stg, in_=w_d[k, :, :])
                nc.vector.tensor_copy(w_sb[:, k, :], stg)
            for k in range(KH):
                stg = xs_pool.tile([128, 4 * H], F32, tag="WSTG")
                nc.sync.dma_start(out=stg, in_=u_d[k, :, :])
                nc.vector.tensor_copy(u_sb[:, k, :], stg)
            if has_b:
                b_sb = consts.tile([1, 4 * H], F32)
                nc.sync.dma_start(out=b_sb, in_=b_d[:, :])

            # ---- initial state ----
            h0_sb = consts.tile([nb, H], F32)
            nc.sync.dma_start(out=h0_sb, in_=h0_d[:, :])
            c_prev = [None, None]
            for n2 in range(2):
                ct = state.tile([32 + nb, NHALF], F16, tag=f"C{n2}")
                nc.sync.dma_start(out=ct[32 : 32 + nb], in_=c0_d[:, n2, :])
                c_prev[n2] = ct

            h0_16 = consts.tile([nb, H], F16)
            nc.vector.tensor_copy(h0_16, h0_sb)
            ht_prev = state.tile([128, KH * nb], MM_DT, tag="HT")
            for n2 in range(2):
                ht0_ps = pt.tile([128, 3 * nb], F16, tag="tcTps")
                for j in range(3):
                    ck = 3 * n2 + j
                    nc.tensor.transpose(
                        ht0_ps[:, j * nb : (j + 1) * nb],
                        h0_16[:, ck * 128 : (ck + 1) * 128],
                        ident16,
                    )
                nc.scalar.copy(
                    out=ht_prev[:, 3 * n2 * nb : (3 * n2 + 3) * nb], in_=ht0_ps
                )

            def load_xblk(t0):
                """DMA + f16-convert the x block starting at step t0."""
                xstg = xs_pool.tile([128, XBLK, KX, nb], F32, tag="XSTG")
                nblk = min(XBLK, t_steps - t0)
                nc.sync.dma_start(
                    out=xstg[:, 0:nblk],
                    in_=xT_d[t0 : t0 + nblk].rearrange("t k p b -> p t k b"),
                )
                xt = xs_pool.tile([128, XBLK, KX, nb], MM_DT, tag="X")
                nc.vector.tensor_copy(xt[:, 0:nblk], xstg[:, 0:nblk])
                return xt

            def x_waves(gates_tile, xt, trel, add_bias):
                """Emit the x-projection waves (accumulation starters)."""
                for n2 in range(2):
                    for k in range(KX):
                        for q in range(4):
                            col = q * H + n2 * NHALF
                            nc.tensor.matmul(
                                gates_tile[32 * q : 32 * q + nb, n2, 0:NHALF],
                                xt[:, trel, k, :],
                                w_sb[:, k, col : col + NHALF],
                                start=(k == 0),
                                stop=False,
                                tile_position=(0, 32 * q),
                                skip_group_check=True,
                            )
                if add_bias:
                    for n2 in range(2):
                        for q, base in ((0, 0), (1, 32), (2, 64), (3, 96)):
                            bq = b_sb[:, q * H + n2 * NHALF : q * H + (n2 + 1) * NHALF]
                            bq = bass.AP(
                                tensor=bq.tensor, offset=bq.offset,
                                ap=[[0, nb]] + bq.ap[1:],
                            )
                            nc.vector.tensor_add(
                                gates_tile[base : base + nb, n2, 0:NHALF],
                                gates_tile[base : base + nb, n2, 0:NHALF],
                                bq,
                            )

            # x block 0 + step-0 x-waves
            x_tile = load_xblk(0)
            gates_cur = pg.tile([128, 2, 512], F32, tag="gates")
            # bias path handled via DVE adds after accumulation finishes, so
            # hoisted x-waves stay start-only; adds are emitted with h-waves.
            x_waves(gates_cur, x_tile, 0, False)

            for t in range(t_steps):
                trel1 = (t + 1) % XBLK
                x_next = x_tile
                if t + 1 < t_steps and trel1 == 0:
                    x_next = load_xblk(t + 1)

                # --- h-waves for step t: h0-dependent chunks (0-2) for both
                # psum halves first, then h1-dependent chunks (3-5) ---
                for ckgrp in range(2):
                    for n2 in range(2):
                        for kc in range(3):
                            ck = 3 * ckgrp + kc
                            for q in range(4):
                                col = q * H + n2 * NHALF
                                nc.tensor.matmul(
                                    gates_cur[32 * q : 32 * q + nb, n2, 0:NHALF],
                                    ht_prev[:, ck * nb : (ck + 1) * nb],
                                    u_sb[:, ck, col : col + NHALF],
                                    start=False,
                                    stop=(ck == KH - 1),
                                    tile_position=(0, 32 * q),
                                    skip_group_check=True,
                                )

                if has_b:
                    for n2 in range(2):
                        for q, base in ((0, 0), (1, 32), (2, 64), (3, 96)):
                            bq = b_sb[:, q * H + n2 * NHALF : q * H + (n2 + 1) * NHALF]
                            bq = bass.AP(
                                tensor=bq.tensor, offset=bq.offset,
                                ap=[[0, nb]] + bq.ap[1:],
                            )
                            nc.vector.tensor_add(
                                gates_cur[base : base + nb, n2, 0:NHALF],
                                gates_cur[base : base + nb, n2, 0:NHALF],
                                bq,
                            )

                # --- next step's x-waves: keep the PE busy during the
                # elementwise phase (HAM stays warm) ---
                gates_next = None
                if t + 1 < t_steps:
                    gates_next = pg.tile([128, 2, 512], F32, tag="gates")
                    x_waves(gates_next, x_next, trel1, False)

                # --- elementwise, halves interleaved stage-by-stage ---
                # S2 rows: i'@0:16, f'@32:48, o'@64:80, sigmoid(2g)@96:112
                S2, C, G, T1, TC = {}, {}, {}, {}, {}
                for n2 in range(2):
                    S2[n2] = gsb.tile([112, NHALF], F16, tag=f"S2{n2}", name=f"S2_{n2}")
                    nc.scalar.activation(
                        out=S2[n2], in_=gates_cur[0:112, n2, 0:NHALF],
                        func=SIG, scale=sc,
                    )
                for n2 in range(2):
                    # g~ = 2*sigmoid(2g) - 1
                    G[n2] = gsb.tile([nb, NHALF], F16, tag=f"G{n2}", name=f"G_{n2}")
                    nc.vector.tensor_scalar(
                        G[n2], S2[n2][96:112], 2.0, -1.0, MULT, ADD
                    )
                for n2 in range(2):
                    # f' * c  (into C rows 32:48)
                    C[n2] = state.tile([32 + nb, NHALF], F16, tag=f"C{n2}", name=f"C_{n2}")
                    nc.vector.tensor_mul(
                        C[n2][32 : 32 + nb], S2[n2][32 : 32 + nb],
                        c_prev[n2][32 : 32 + nb],
                    )
                for n2 in range(2):
                    # i' * g~ (out-shift to rows 32:48)
                    T1[n2] = ew.tile([32 + nb, NHALF], F16, tag=f"T1{n2}", name=f"T1_{n2}")
                    nc.vector.tensor_mul(
                        T1[n2][32 : 32 + nb], S2[n2][0:nb], G[n2]
                    )
                for n2 in range(2):
                    nc.vector.tensor_add(
                        C[n2][32 : 32 + nb], C[n2][32 : 32 + nb],
                        T1[n2][32 : 32 + nb],
                    )
                # o'^T: transpose o' into psum then DVE-copy to SBUF (off the
                # critical chain - o' is ready right after the sigmoid)
                oT_ps, oT = {}, {}
                for n2 in range(2):
                    oT_ps[n2] = pt.tile([128, 3 * nb], F16, tag="oTps", name=f"oTps_{n2}")
                    for j in range(3):
                        nc.tensor.transpose(
                            oT_ps[n2][:, j * nb : (j + 1) * nb],
                            S2[n2][64 : 64 + nb, j * 128 : (j + 1) * 128],
                            idento[64 : 64 + nb],
                        )
                for n2 in range(2):
                    TC[n2] = ew.tile([64 + nb, NHALF], F16, tag=f"TC{n2}", name=f"TC_{n2}")
                    nc.scalar.activation(
                        out=TC[n2][64 : 64 + nb], in_=C[n2][32 : 32 + nb],
                        func=TANH,
                    )
                for n2 in range(2):
                    oT[n2] = ew.tile([128, 3 * nb], F16, tag=f"oT{n2}", name=f"oT_{n2}")
                    nc.vector.tensor_copy(oT[n2], oT_ps[n2])
                # tc^T via PE transpose (stays in psum), then
                # h^T = o'^T * tc^T written directly into the stationary tile
                ht_new = state.tile([128, KH * nb], MM_DT, tag="HT")
                for n2 in range(2):
                    tcT_ps = pt.tile([128, 3 * nb], F16, tag="tcTps", name=f"tcTps_{n2}")
                    for j in range(3):
                        nc.tensor.transpose(
                            tcT_ps[:, j * nb : (j + 1) * nb],
                            TC[n2][64 : 64 + nb, j * 128 : (j + 1) * 128],
                            idento[64 : 64 + nb],
                        )
                    nc.vector.tensor_mul(
                        ht_new[:, 3 * n2 * nb : (3 * n2 + 3) * nb],
                        oT[n2], tcT_ps,
                    )
                for n2 in range(2):
                    nc.sync.dma_start(
                        out=cs_d[:, t, n2], in_=C[n2][32 : 32 + nb]
                    )
                    c_prev[n2] = C[n2]
                nc.sync.dma_start(out=hs_d[t, :, :], in_=ht_new)

                ht_prev = ht_new
                gates_cur = gates_next
                x_tile = x_next

    nc.finalize()
    return nc


# Column permutation: reference gate order (i, f, g~, o) -> kernel (i, f, o, g~)
def _gate_perm():
    return np.concatenate(
        [np.arange(0, H), np.arange(H, 2 * H), np.arange(3 * H, 4 * H),
         np.arange(2 * H, 3 * H)]
    )


def _prep_core_inputs(input_, h0, c0, Wp, Up, bp, t_steps):
    nb = input_.shape[0]
    xT = np.ascontiguousarray(
        input_[:, :t_steps].transpose(1, 2, 0).reshape(t_steps, KX, 128, nb)
    )
    return {
        "xT": xT,
        "h0": np.ascontiguousarray(h0),
        "c0": np.ascontiguousarray(c0.reshape(nb, 2, NHALF).astype(np.float16)),
        "w": Wp,
        "u": Up,
        "b": bp,
    }


def run(input, hiddenState, cellState, W, U, b, t_steps=T, trace=False):
    input = np.asarray(input, np.float32)
    hiddenState = np.asarray(hiddenState, np.float32)
    cellState = np.asarray(cellState, np.float32)
    W = np.asarray(W, np.float32)
    U = np.asarray(U, np.float32)
    b = np.asarray(b, np.float32)

    perm = _gate_perm()
    Wp = np.ascontiguousarray(W[:, perm].reshape(KX, 128, 4 * H))
    Up = np.ascontiguousarray(U[:, perm].reshape(KH, 128, 4 * H))
    bp = np.ascontiguousarray(b[perm].reshape(1, 4 * H))
    has_b = bool(np.any(b))

    nc = build_lstm(NB, t_steps, has_b)
    in_maps = []
    for c in range(NCORES):
        bs = slice(c * NB, (c + 1) * NB)
        in_maps.append(
            _prep_core_inputs(
                input[bs], hiddenState[bs], cellState[bs], Wp, Up, bp, t_steps
            )
        )
    res = run_bass_kernel_spmd(
        nc, in_maps, core_ids=list(range(NCORES)), trace=trace
    )

    hs = np.empty((B, t_steps, H), np.float32)
    cs = np.empty((B, t_steps, H), np.float32)
    for c in range(NCORES):
        bs = slice(c * NB, (c + 1) * NB)
        ht = res.results[c]["hs"].astype(np.float32)  # [t, 128, 6*16]
        ht = ht.reshape(t_steps, 128, KH, NB)
        hs[bs] = ht.transpose(3, 0, 2, 1).reshape(NB, t_steps, H)
        cs[bs] = res.results[c]["cs"].astype(np.float32).reshape(NB, t_steps, H)
    return (hs, cs), res


def kernel(input, hiddenState, cellState, W, U, b):
    (hs, cs), _ = run(input, hiddenState, cellState, W, U, b)
    return hs, cs


# revision 8
# speedup vs baseline: 1.2853x; 1.2056x over previous
"""LSTM (BaseRNN) Trainium2 kernel.

Problem: B=128, T=512, I=256, H=768 LSTM; returns (hiddenStates, cellStates)
each [B, T, H] fp32.

Strategy (data-parallel over batch, 8 cores x 16 rows):
  - Batch-major gate preactivations g_t = x_t W + h_{t-1} U accumulated in
    PSUM as [batch, gate_cols]; stationary operand = x^T / h^T chunks
    [128, 16] fp16, moving operand = W/U chunks [128, 384] fp16.
  - 4-way PE col-group tiling: gate q -> col group q (psum partitions
    32q..); every group's first matmul carries start=True.
  - Host permutes gate columns to (i, f, o, g~).  One ACT sigmoid with a
    per-partition scale vector (1 for i/f/o rows, 2 for g~ rows) covers all
    four gates; tanh(x) = 2*sigmoid(2x)-1 is reconstructed by a cheap DVE
    tensor_scalar.
  - The step is processed in two independent column halves (psum banks);
    gate values, c, and h are fp16.
  - Pipelining for latency (the recurrent cycle is the bound):
      * next step's x-waves are emitted right after this step's h-waves so
        the PE stays busy (HAM stays warm) during the elementwise phase;
      * h-waves are ordered [h0-chunks psum0, h0-chunks psum1, h1-chunks
        psum0, h1-chunks psum1] so each half's sigmoid fires as early as
        possible;
      * G is written into spare partitions of the c_prev tile so f'*c and
        i'*g~ fuse into one [48,384] DVE multiply;
      * oT psum->sbuf copies run on DVE (ACT does only sigmoid+tanh).
  - h_t half is re-transposed via 3 PE-transposes into the fp16 h^T
    stationary for the next step.
  - hs/cs stream to DRAM as fp16; the host upcasts to fp32.
"""

import numpy as np

import concourse.bass as bass
import concourse.bacc as bacc
import concourse.tile as tile
from concourse import mybir
from concourse.bass_utils import run_bass_kernel_spmd
from concourse.masks import make_identity

B, T, I, H = 128, 512, 256, 768
NCORES = 8
NB = B // NCORES  # 16
KX = I // 128  # 2 x chunks
KH = H // 128  # 6 h chunks
NK = KX + KH  # 8 contraction waves
NHALF = H // 2  # 384: per-gate psum half (one matmul's N)
F32 = mybir.dt.float32
F16 = mybir.dt.float16
XBLK = 32  # x-stream block (steps per DMA)

MM_DT = F16  # matmul operand dtype


def build_lstm(nb=NB, t_steps=T, has_b=False):
    nc = bacc.Bacc(None, target_bir_lowering=False)

    xT_d = nc.dram_tensor("xT", [t_steps, KX, 128, nb], F32, kind="ExternalInput")
    h0_d = nc.dram_tensor("h0", [nb, H], F32, kind="ExternalInput")
    c0_d = nc.dram_tensor("c0", [nb, 2, NHALF], F16, kind="ExternalInput")
    w_d = nc.dram_tensor("w", [KX, 128, 4 * H], F32, kind="ExternalInput")
    u_d = nc.dram_tensor("u", [KH, 128, 4 * H], F32, kind="ExternalInput")
    b_d = nc.dram_tensor("b", [1, 4 * H], F32, kind="ExternalInput")
    hs_d = nc.dram_tensor("hs", [t_steps, 128, KH * NB], F16, kind="ExternalOutput")
    cs_d = nc.dram_tensor("cs", [nb, t_steps, 2, NHALF], F16, kind="ExternalOutput")

    SIG = mybir.ActivationFunctionType.Sigmoid
    TANH = mybir.ActivationFunctionType.Tanh
    MULT = mybir.AluOpType.mult
    ADD = mybir.AluOpType.add

    with tile.TileContext(nc) as tc:
        with (
            tc.tile_pool(name="consts", bufs=1) as consts,
            tc.tile_pool(name="xs", bufs=2) as xs_pool,
            tc.tile_pool(name="gsb", bufs=3) as gsb,
            tc.tile_pool(name="ew", bufs=3) as ew,
            tc.tile_pool(name="state", bufs=2) as state,
            tc.tile_pool(name="pg", bufs=2, space="PSUM") as pg,
            tc.tile_pool(name="pt", bufs=2, space="PSUM") as pt,
        ):
            ident16 = consts.tile([nb, nb], F16)
            make_identity(nc, ident16)
            idento = consts.tile([64 + nb, nb], F16)
            make_identity(nc, idento[64 : 64 + nb])
            # per-partition activation input scale: 1 for i/f/o rows,
            # 2 for g~ rows (tanh(x) = 2*sigmoid(2x) - 1)
            sc = consts.tile([112, 1], F32)
            nc.vector.memset(sc[0:96], 1.0)
            nc.vector.memset(sc[96:112], 2.0)

            # weights: DMA f32 staging -> round-convert to fp16
            w_sb = consts.tile([128, KX, 4 * H], MM_DT)
            u_sb = consts.tile([128, KH, 4 * H], MM_DT)
            for k in range(KX):
                stg = xs_pool.tile([128, 4 * H], F32, tag="WSTG")
                nc.sync.dma_start(out=stg, in_=w_d[k, :, :])
                nc.vector.tensor_copy(w_sb[:, k, :], stg)
            for k in range(KH):
                stg = xs_pool.tile([128, 4 * H], F32, tag="WSTG")
                nc.sync.dma_start(out=stg, in_=u_d[k, :, :])
                nc.vector.tensor_copy(u_sb[:, k, :], stg)
            if has_b:
                b_sb = consts.tile([1, 4 * H], F32)
                nc.sync.dma_start(out=b_sb, in_=b_d[:, :])

            # ---- initial state ----
            h0_sb = consts.tile([nb, H], F32)
            nc.sync.dma_start(out=h0_sb, in_=h0_d[:, :])
            c_prev = [None, None]
            for n2 in range(2):
                ct = state.tile([32 + nb, NHALF], F16, tag=f"C{n2}")
                nc.sync.dma_start(out=ct[32 : 32 + nb], in_=c0_d[:, n2, :])
                c_prev[n2] = ct

            h0_16 = consts.tile([nb, H], F16)
            nc.vector.tensor_copy(h0_16, h0_sb)
            ht_prev = state.tile([128, KH * nb], MM_DT, tag="HT")
            for n2 in range(2):
                ht0_ps = pt.tile([128, 3 * nb], F16, tag="tcTps")
                for j in range(3):
                    ck = 3 * n2 + j
                    nc.tensor.transpose(
                        ht0_ps[:, j * nb : (j + 1) * nb],
                        h0_16[:, ck * 128 : (ck + 1) * 128],
                        ident16,
                    )
                nc.scalar.copy(
                    out=ht_prev[:, 3 * n2 * nb : (3 * n2 + 3) * nb], in_=ht0_ps
                )

            def load_xblk(t0):
                """DMA + f16-convert the x block starting at step t0."""
                xstg = xs_pool.tile([128, XBLK, KX, nb], F32, tag="XSTG")
                nblk = min(XBLK, t_steps - t0)
                nc.sync.dma_start(
                    out=xstg[:, 0:nblk],
                    in_=xT_d[t0 : t0 + nblk].rearrange("t k p b -> p t k b"),
                )
                xt = xs_pool.tile([128, XBLK, KX, nb], MM_DT, tag="X")
                nc.vector.tensor_copy(xt[:, 0:nblk], xstg[:, 0:nblk])
                return xt

            def x_waves(gates_tile, xt, trel, add_bias):
                """Emit the x-projection waves (accumulation starters)."""
                for n2 in range(2):
                    for k in range(KX):
                        for q in range(4):
                            col = q * H + n2 * NHALF
                            nc.tensor.matmul(
                                gates_tile[32 * q : 32 * q + nb, n2, 0:NHALF],
                                xt[:, trel, k, :],
                                w_sb[:, k, col : col + NHALF],
                                start=(k == 0),
                                stop=False,
                                tile_position=(0, 32 * q),
                                skip_group_check=True,
                            )
                if add_bias:
                    for n2 in range(2):
                        for q, base in ((0, 0), (1, 32), (2, 64), (3, 96)):
                            bq = b_sb[:, q * H + n2 * NHALF : q * H + (n2 + 1) * NHALF]
                            bq = bass.AP(
                                tensor=bq.tensor, offset=bq.offset,
                                ap=[[0, nb]] + bq.ap[1:],
                            )
                            nc.vector.tensor_add(
                                gates_tile[base : base + nb, n2, 0:NHALF],
                                gates_tile[base : base + nb, n2, 0:NHALF],
                                bq,
                            )

            # x block 0 + step-0 x-waves
            x_tile = load_xblk(0)
            gates_cur = pg.tile([128, 2, 512], F32, tag="gates")
            # bias path handled via DVE adds after accumulation finishes, so
            # hoisted x-waves stay start-only; adds are emitted with h-waves.
            x_waves(gates_cur, x_tile, 0, False)

            for t in range(t_steps):
                trel1 = (t + 1) % XBLK
                x_next = x_tile
                if t + 1 < t_steps and trel1 == 0:
                    x_next = load_xblk(t + 1)

                # --- h-waves for step t: h0-dependent chunks (0-2) for both
                # psum halves first, then h1-dependent chunks (3-5) ---
                for ckgrp in range(2):
                    for n2 in range(2):
                        for kc in range(3):
                            ck = 3 * ckgrp + kc
                            for q in range(4):
                                col = q * H + n2 * NHALF
                                nc.tensor.matmul(
                                    gates_cur[32 * q : 32 * q + nb, n2, 0:NHALF],
                                    ht_prev[:, ck * nb : (ck + 1) * nb],
                                    u_sb[:, ck, col : col + NHALF],
                                    start=False,
                                    stop=(ck == KH - 1),
                                    tile_position=(0, 32 * q),
                                    skip_group_check=True,
                                )

                if has_b:
                    for n2 in range(2):
                        for q, base in ((0, 0), (1, 32), (2, 64), (3, 96)):
                            bq = b_sb[:, q * H + n2 * NHALF : q * H + (n2 + 1) * NHALF]
                            bq = bass.AP(
                                tensor=bq.tensor, offset=bq.offset,
                                ap=[[0, nb]] + bq.ap[1:],
                            )
                            nc.vector.tensor_add(
                                gates_cur[base : base + nb, n2, 0:NHALF],
                                gates_cur[base : base + nb, n2, 0:NHALF],
                                bq,
                            )

                # --- next step's x-waves: keep the PE busy during the
                # elementwise phase (HAM stays warm) ---
                gates_next = None
                if t + 1 < t_steps:
                    gates_next = pg.tile([128, 2, 512], F32, tag="gates")
                    x_waves(gates_next, x_next, trel1, False)

                def dummy_mms(n):
                    # Warm-keeper matmuls into the unused 384:512 column
                    # region of the next gate psum tile: the PE's HAM clock
                    # gate re-throttles to 1.2 GHz whenever the PE idles for
                    # part of a 4096-cycle window, which would make every
                    # wave ~2x slower. These fillers run during the
                    # elementwise-phase dependency stalls.
                    if gates_next is None:
                        return
                    for i in range(n):
                        nc.tensor.matmul(
                            gates_next[0:nb, i % 2, 384:512],
                            ht_prev[:, 0:nb],
                            u_sb[:, 0, 0:128],
                            start=False,
                            stop=False,
                            tile_position=(0, 0),
                            skip_group_check=True,
                        )

                dummy_mms(8)

                # --- elementwise, halves interleaved stage-by-stage ---
                # S2 rows: i'@0:16, f'@32:48, o'@64:80, sigmoid(2g)@96:112
                S2, C, G, T1, TC = {}, {}, {}, {}, {}
                for n2 in range(2):
                    S2[n2] = gsb.tile([112, NHALF], F16, tag=f"S2{n2}", name=f"S2_{n2}")
                    nc.scalar.activation(
                        out=S2[n2], in_=gates_cur[0:112, n2, 0:NHALF],
                        func=SIG, scale=sc,
                    )
                for n2 in range(2):
                    # g~ = 2*sigmoid(2g) - 1
                    G[n2] = gsb.tile([nb, NHALF], F16, tag=f"G{n2}", name=f"G_{n2}")
                    nc.vector.tensor_scalar(
                        G[n2], S2[n2][96:112], 2.0, -1.0, MULT, ADD
                    )
                for n2 in range(2):
                    # f' * c  (into C rows 32:48)
                    C[n2] = state.tile([32 + nb, NHALF], F16, tag=f"C{n2}", name=f"C_{n2}")
                    nc.vector.tensor_mul(
                        C[n2][32 : 32 + nb], S2[n2][32 : 32 + nb],
                        c_prev[n2][32 : 32 + nb],
                    )
                for n2 in range(2):
                    # i' * g~ (out-shift to rows 32:48)
                    T1[n2] = ew.tile([32 + nb, NHALF], F16, tag=f"T1{n2}", name=f"T1_{n2}")
                    nc.vector.tensor_mul(
                        T1[n2][32 : 32 + nb], S2[n2][0:nb], G[n2]
                    )
                for n2 in range(2):
                    nc.vector.tensor_add(
                        C[n2][32 : 32 + nb], C[n2][32 : 32 + nb],
                        T1[n2][32 : 32 + nb],
                    )
                # o'^T: transpose o' into psum then DVE-copy to SBUF (off the
                # critical chain - o' is ready right after the sigmoid)
                oT_ps, oT = {}, {}
                for n2 in range(2):
                    oT_ps[n2] = pt.tile([128, 3 * nb], F16, tag="oTps", name=f"oTps_{n2}")
                    for j in range(3):
                        nc.tensor.transpose(
                            oT_ps[n2][:, j * nb : (j + 1) * nb],
                            S2[n2][64 : 64 + nb, j * 128 : (j + 1) * 128],
                            idento[64 : 64 + nb],
                        )
                dummy_mms(8)
                for n2 in range(2):
                    TC[n2] = ew.tile([64 + nb, NHALF], F16, tag=f"TC{n2}", name=f"TC_{n2}")
                    nc.scalar.activation(
                        out=TC[n2][64 : 64 + nb], in_=C[n2][32 : 32 + nb],
                        func=TANH,
                    )
                for n2 in range(2):
                    oT[n2] = ew.tile([128, 3 * nb], F16, tag=f"oT{n2}", name=f"oT_{n2}")
                    nc.vector.tensor_copy(oT[n2], oT_ps[n2])
                # tc^T via PE transpose (stays in psum), then
                # h^T = o'^T * tc^T written directly into the stationary tile
                ht_new = state.tile([128, KH * nb], MM_DT, tag="HT")
                for n2 in range(2):
                    tcT_ps = pt.tile([128, 3 * nb], F16, tag="tcTps", name=f"tcTps_{n2}")
                    for j in range(3):
                        nc.tensor.transpose(
                            tcT_ps[:, j * nb : (j + 1) * nb],
                            TC[n2][64 : 64 + nb, j * 128 : (j + 1) * 128],
                            idento[64 : 64 + nb],
                        )
                    nc.vector.tensor_mul(
                        ht_new[:, 3 * n2 * nb : (3 * n2 + 3) * nb],
                        oT[n2], tcT_ps,
                    )
                for n2 in range(2):
                    nc.sync.dma_start(
                        out=cs_d[:, t, n2], in_=C[n2][32 : 32 + nb]
                    )
                    c_prev[n2] = C[n2]
                nc.sync.dma_start(out=hs_d[t, :, :], in_=ht_new)

                ht_prev = ht_new
                gates_cur = gates_next
                x_tile = x_next

    nc.finalize()
    return nc


# Column permutation: reference gate order (i, f, g~, o) -> kernel (i, f, o, g~)
def _gate_perm():
    return np.concatenate(
        [np.arange(0, H), np.arange(H, 2 * H), np.arange(3 * H, 4 * H),
         np.arange(2 * H, 3 * H)]
    )


def _prep_core_inputs(input_, h0, c0, Wp, Up, bp, t_steps):
    nb = input_.shape[0]
    xT = np.ascontiguousarray(
        input_[:, :t_steps].transpose(1, 2, 0).reshape(t_steps, KX, 128, nb)
    )
    return {
        "xT": xT,
        "h0": np.ascontiguousarray(h0),
        "c0": np.ascontiguousarray(c0.reshape(nb, 2, NHALF).astype(np.float16)),
        "w": Wp,
        "u": Up,
        "b": bp,
    }


def run(input, hiddenState, cellState, W, U, b, t_steps=T, trace=False):
    input = np.asarray(input, np.float32)
    hiddenState = np.asarray(hiddenState, np.float32)
    cellState = np.asarray(cellState, np.float32)
    W = np.asarray(W, np.float32)
    U = np.asarray(U, np.float32)
    b = np.asarray(b, np.float32)

    perm = _gate_perm()
    Wp = np.ascontiguousarray(W[:, perm].reshape(KX, 128, 4 * H))
    Up = np.ascontiguousarray(U[:, perm].reshape(KH, 128, 4 * H))
    bp = np.ascontiguousarray(b[perm].reshape(1, 4 * H))
    has_b = bool(np.any(b))

    nc = build_lstm(NB, t_steps, has_b)
    in_maps = []
    for c in range(NCORES):
        bs = slice(c * NB, (c + 1) * NB)
        in_maps.append(
            _prep_core_inputs(
                input[bs], hiddenState[bs], cellState[bs], Wp, Up, bp, t_steps
            )
        )
    res = run_bass_kernel_spmd(
        nc, in_maps, core_ids=list(range(NCORES)), trace=trace
    )

    hs = np.empty((B, t_steps, H), np.float32)
    cs = np.empty((B, t_steps, H), np.float32)
    for c in range(NCORES):
        bs = slice(c * NB, (c + 1) * NB)
        ht = res.results[c]["hs"].astype(np.float32)  # [t, 128, 6*16]
        ht = ht.reshape(t_steps, 128, KH, NB)
        hs[bs] = ht.transpose(3, 0, 2, 1).reshape(NB, t_steps, H)
        cs[bs] = res.results[c]["cs"].astype(np.float32).reshape(NB, t_steps, H)
    return (hs, cs), res


def kernel(input, hiddenState, cellState, W, U, b):
    (hs, cs), _ = run(input, hiddenState, cellState, W, U, b)
    return hs, cs


# revision 11
# speedup vs baseline: 1.3072x; 1.0170x over previous
"""LSTM (BaseRNN) Trainium2 kernel.

Problem: B=128, T=512, I=256, H=768 LSTM; returns (hiddenStates, cellStates)
each [B, T, H] fp32.

Strategy (data-parallel over batch, 8 cores x 16 rows):
  - Batch-major gate preactivations g_t = x_t W + h_{t-1} U accumulated in
    PSUM as [batch, gate_cols]; stationary operand = x^T / h^T chunks
    [128, 16] fp16, moving operand = W/U chunks [128, 384] fp16.
  - 4-way PE col-group tiling: gate q -> col group q (psum partitions
    32q..); every group's first matmul carries start=True.
  - Host permutes gate columns to (i, f, o, g~).  One ACT sigmoid with a
    per-partition scale vector (1 for i/f/o rows, 2 for g~ rows) covers all
    four gates; tanh(x) = 2*sigmoid(2x)-1 is reconstructed by a cheap DVE
    tensor_scalar.
  - The step is processed in two independent column halves (psum banks);
    gate values, c, and h are fp16.
  - Pipelining for latency (the recurrent cycle is the bound):
      * next step's x-waves are emitted right after this step's h-waves so
        the PE stays busy (HAM stays warm) during the elementwise phase;
      * h-waves are ordered [h0-chunks psum0, h0-chunks psum1, h1-chunks
        psum0, h1-chunks psum1] so each half's sigmoid fires as early as
        possible;
      * G is written into spare partitions of the c_prev tile so f'*c and
        i'*g~ fuse into one [48,384] DVE multiply;
      * oT psum->sbuf copies run on DVE (ACT does only sigmoid+tanh).
  - h_t half is re-transposed via 3 PE-transposes into the fp16 h^T
    stationary for the next step.
  - hs/cs stream to DRAM as fp16; the host upcasts to fp32.
"""

import numpy as np

import concourse.bass as bass
import concourse.bacc as bacc
import concourse.tile as tile
from concourse import mybir
from concourse.bass_utils import run_bass_kernel_spmd
from concourse.masks import make_identity

B, T, I, H = 128, 512, 256, 768
NCORES = 8
NB = B // NCORES  # 16
KX = I // 128  # 2 x chunks
KH = H // 128  # 6 h chunks
NK = KX + KH  # 8 contraction waves
NHALF = H // 2  # 384: per-gate psum half (one matmul's N)
F32 = mybir.dt.float32
F16 = mybir.dt.float16
XBLK = 32  # x-stream block (steps per DMA)

MM_DT = F16  # matmul operand dtype


def build_lstm(nb=NB, t_steps=T, has_b=False):
    nc = bacc.Bacc(None, target_bir_lowering=False)

    xT_d = nc.dram_tensor("xT", [t_steps, KX, 128, nb], F32, kind="ExternalInput")
    h0_d = nc.dram_tensor("h0", [nb, H], F32, kind="ExternalInput")
    c0_d = nc.dram_tensor("c0", [nb, 2, NHALF], F16, kind="ExternalInput")
    w_d = nc.dram_tensor("w", [KX, 128, 4 * H], F32, kind="ExternalInput")
    u_d = nc.dram_tensor("u", [KH, 128, 4 * H], F32, kind="ExternalInput")
    b_d = nc.dram_tensor("b", [1, 4 * H], F32, kind="ExternalInput")
    hs_d = nc.dram_tensor("hs", [t_steps, 128, KH * NB], F16, kind="ExternalOutput")
    cs_d = nc.dram_tensor("cs", [nb, t_steps, 2, NHALF], F16, kind="ExternalOutput")

    SIG = mybir.ActivationFunctionType.Sigmoid
    TANH = mybir.ActivationFunctionType.Tanh
    MULT = mybir.AluOpType.mult
    ADD = mybir.AluOpType.add

    with tile.TileContext(nc) as tc:
        with (
            tc.tile_pool(name="consts", bufs=1) as consts,
            tc.tile_pool(name="xs", bufs=2) as xs_pool,
            tc.tile_pool(name="gsb", bufs=3) as gsb,
            tc.tile_pool(name="ew", bufs=3) as ew,
            tc.tile_pool(name="state", bufs=2) as state,
            tc.tile_pool(name="pg", bufs=2, space="PSUM") as pg,
            tc.tile_pool(name="pt", bufs=2, space="PSUM") as pt,
        ):
            ident16 = consts.tile([nb, nb], F16)
            make_identity(nc, ident16)
            idento = consts.tile([64 + nb, nb], F16)
            make_identity(nc, idento[64 : 64 + nb])
            # per-partition activation input scale: 1 for i/f/o rows,
            # 2 for g~ rows (tanh(x) = 2*sigmoid(2x) - 1)
            sc = consts.tile([112, 1], F32)
            nc.vector.memset(sc[0:96], 1.0)
            nc.vector.memset(sc[96:112], 2.0)

            # weights: DMA f32 staging -> round-convert to fp16
            w_sb = consts.tile([128, KX, 4 * H], MM_DT)
            u_sb = consts.tile([128, KH, 4 * H], MM_DT)
            for k in range(KX):
                stg = xs_pool.tile([128, 4 * H], F32, tag="WSTG")
                nc.sync.dma_start(out=stg, in_=w_d[k, :, :])
                nc.vector.tensor_copy(w_sb[:, k, :], stg)
            for k in range(KH):
                stg = xs_pool.tile([128, 4 * H], F32, tag="WSTG")
                nc.sync.dma_start(out=stg, in_=u_d[k, :, :])
                nc.vector.tensor_copy(u_sb[:, k, :], stg)
            if has_b:
                b_sb = consts.tile([1, 4 * H], F32)
                nc.sync.dma_start(out=b_sb, in_=b_d[:, :])

            # ---- initial state ----
            h0_sb = consts.tile([nb, H], F32)
            nc.sync.dma_start(out=h0_sb, in_=h0_d[:, :])
            c_prev = [None, None]
            for n2 in range(2):
                ct = state.tile([32 + nb, NHALF], F16, tag=f"C{n2}")
                nc.sync.dma_start(out=ct[32 : 32 + nb], in_=c0_d[:, n2, :])
                c_prev[n2] = ct

            h0_16 = consts.tile([nb, H], F16)
            nc.vector.tensor_copy(h0_16, h0_sb)
            ht_prev = state.tile([128, KH * nb], MM_DT, tag="HT")
            for n2 in range(2):
                ht0_ps = pt.tile([128, 3 * nb], F16, tag="tcTps")
                for j in range(3):
                    ck = 3 * n2 + j
                    nc.tensor.transpose(
                        ht0_ps[:, j * nb : (j + 1) * nb],
                        h0_16[:, ck * 128 : (ck + 1) * 128],
                        ident16,
                    )
                nc.scalar.copy(
                    out=ht_prev[:, 3 * n2 * nb : (3 * n2 + 3) * nb], in_=ht0_ps
                )

            def load_xblk(t0):
                """DMA + f16-convert the x block starting at step t0."""
                xstg = xs_pool.tile([128, XBLK, KX, nb], F32, tag="XSTG")
                nblk = min(XBLK, t_steps - t0)
                nc.sync.dma_start(
                    out=xstg[:, 0:nblk],
                    in_=xT_d[t0 : t0 + nblk].rearrange("t k p b -> p t k b"),
                )
                xt = xs_pool.tile([128, XBLK, KX, nb], MM_DT, tag="X")
                nc.vector.tensor_copy(xt[:, 0:nblk], xstg[:, 0:nblk])
                return xt

            def x_waves(gates_tile, xt, trel, add_bias):
                """Emit the x-projection waves (accumulation starters)."""
                for n2 in range(2):
                    for k in range(KX):
                        for q in range(4):
                            col = q * H + n2 * NHALF
                            nc.tensor.matmul(
                                gates_tile[32 * q : 32 * q + nb, n2, 0:NHALF],
                                xt[:, trel, k, :],
                                w_sb[:, k, col : col + NHALF],
                                start=(k == 0),
                                stop=False,
                                tile_position=(0, 32 * q),
                                skip_group_check=True,
                            )
                if add_bias:
                    for n2 in range(2):
                        for q, base in ((0, 0), (1, 32), (2, 64), (3, 96)):
                            bq = b_sb[:, q * H + n2 * NHALF : q * H + (n2 + 1) * NHALF]
                            bq = bass.AP(
                                tensor=bq.tensor, offset=bq.offset,
                                ap=[[0, nb]] + bq.ap[1:],
                            )
                            nc.vector.tensor_add(
                                gates_tile[base : base + nb, n2, 0:NHALF],
                                gates_tile[base : base + nb, n2, 0:NHALF],
                                bq,
                            )

            # x block 0 + step-0 x-waves
            x_tile = load_xblk(0)
            gates_cur = pg.tile([128, 2, 512], F32, tag="gates")
            # bias path handled via DVE adds after accumulation finishes, so
            # hoisted x-waves stay start-only; adds are emitted with h-waves.
            x_waves(gates_cur, x_tile, 0, False)

            for t in range(t_steps):
                trel1 = (t + 1) % XBLK
                x_next = x_tile
                if t + 1 < t_steps and trel1 == 0:
                    x_next = load_xblk(t + 1)

                # --- h-waves for step t: h0-dependent chunks (0-2) for both
                # psum halves first, then h1-dependent chunks (3-5) ---
                for ckgrp in range(2):
                    for n2 in range(2):
                        for kc in range(3):
                            ck = 3 * ckgrp + kc
                            for q in range(4):
                                col = q * H + n2 * NHALF
                                nc.tensor.matmul(
                                    gates_cur[32 * q : 32 * q + nb, n2, 0:NHALF],
                                    ht_prev[:, ck * nb : (ck + 1) * nb],
                                    u_sb[:, ck, col : col + NHALF],
                                    start=False,
                                    stop=(ck == KH - 1),
                                    tile_position=(0, 32 * q),
                                    skip_group_check=True,
                                )

                if has_b:
                    for n2 in range(2):
                        for q, base in ((0, 0), (1, 32), (2, 64), (3, 96)):
                            bq = b_sb[:, q * H + n2 * NHALF : q * H + (n2 + 1) * NHALF]
                            bq = bass.AP(
                                tensor=bq.tensor, offset=bq.offset,
                                ap=[[0, nb]] + bq.ap[1:],
                            )
                            nc.vector.tensor_add(
                                gates_cur[base : base + nb, n2, 0:NHALF],
                                gates_cur[base : base + nb, n2, 0:NHALF],
                                bq,
                            )

                # --- next step's x-waves: keep the PE busy during the
                # elementwise phase (HAM stays warm) ---
                gates_next = None
                if t + 1 < t_steps:
                    gates_next = pg.tile([128, 2, 512], F32, tag="gates")
                    x_waves(gates_next, x_next, trel1, False)

                def dummy_mms(n):
                    # Warm-keeper matmuls into the unused 384:512 column
                    # region of the next gate psum tile: the PE's HAM clock
                    # gate re-throttles to 1.2 GHz whenever the PE idles for
                    # part of a 4096-cycle window, which would make every
                    # wave ~2x slower. These fillers run during the
                    # elementwise-phase dependency stalls.
                    if gates_next is None:
                        return
                    for i in range(n):
                        nc.tensor.matmul(
                            gates_next[0:nb, i % 2, 384:512],
                            ht_prev[:, 0:nb],
                            u_sb[:, 0, 0:128],
                            start=False,
                            stop=False,
                            tile_position=(0, 0),
                            skip_group_check=True,
                        )

                dummy_mms(6)

                # --- elementwise, half-major: half0's whole DVE chain is
                # emitted (and runs) before half1's, so half0's tail
                # (tanh/transpose/h^T) overlaps half1's DVE work and the
                # next step's h0-chunk waves start as early as possible ---
                # S2 rows: i'@0:16, f'@32:48, o'@64:80, sigmoid(2g)@96:112
                S2, C, G, T1, TC = {}, {}, {}, {}, {}
                for n2 in range(2):
                    S2[n2] = gsb.tile([112, NHALF], F16, tag=f"S2{n2}", name=f"S2_{n2}")
                    nc.scalar.activation(
                        out=S2[n2], in_=gates_cur[0:112, n2, 0:NHALF],
                        func=SIG, scale=sc,
                    )
                oT_ps, oT, tcT_ps = {}, {}, {}
                ht_new = state.tile([128, KH * nb], MM_DT, tag="HT")
                for n2 in range(2):
                    # g~ = 2*sigmoid(2g) - 1
                    G[n2] = gsb.tile([nb, NHALF], F16, tag=f"G{n2}", name=f"G_{n2}")
                    nc.vector.tensor_scalar(
                        G[n2], S2[n2][96:112], 2.0, -1.0, MULT, ADD
                    )
                    # f' * c  (into C rows 32:48)
                    C[n2] = state.tile([32 + nb, NHALF], F16, tag=f"C{n2}", name=f"C_{n2}")
                    nc.vector.tensor_mul(
                        C[n2][32 : 32 + nb], S2[n2][32 : 32 + nb],
                        c_prev[n2][32 : 32 + nb],
                    )
                    # i' * g~ (out-shift to rows 32:48)
                    T1[n2] = ew.tile([32 + nb, NHALF], F16, tag=f"T1{n2}", name=f"T1_{n2}")
                    nc.vector.tensor_mul(
                        T1[n2][32 : 32 + nb], S2[n2][0:nb], G[n2]
                    )
                    nc.vector.tensor_add(
                        C[n2][32 : 32 + nb], C[n2][32 : 32 + nb],
                        T1[n2][32 : 32 + nb],
                    )
                    # o'^T into psum (PE), then ACT copies it to SBUF (ACT
                    # has slack during the DVE chain; keeps DVE queue clear)
                    oT_ps[n2] = pt.tile([128, 3 * nb], F16, tag="oTps", name=f"oTps_{n2}")
                    for j in range(3):
                        nc.tensor.transpose(
                            oT_ps[n2][:, j * nb : (j + 1) * nb],
                            S2[n2][64 : 64 + nb, j * 128 : (j + 1) * 128],
                            idento[64 : 64 + nb],
                        )
                    oT[n2] = ew.tile([128, 3 * nb], F16, tag=f"oT{n2}", name=f"oT_{n2}")
                    nc.scalar.copy(out=oT[n2], in_=oT_ps[n2])
                    TC[n2] = ew.tile([64 + nb, NHALF], F16, tag=f"TC{n2}", name=f"TC_{n2}")
                    nc.scalar.activation(
                        out=TC[n2][64 : 64 + nb], in_=C[n2][32 : 32 + nb],
                        func=TANH,
                    )
                    if n2 == 0:
                        dummy_mms(6)
                    # tc^T via PE transpose (stays in psum)
                    tcT_ps[n2] = pt.tile([128, 3 * nb], F16, tag="tcTps", name=f"tcTps_{n2}")
                    for j in range(3):
                        nc.tensor.transpose(
                            tcT_ps[n2][:, j * nb : (j + 1) * nb],
                            TC[n2][64 : 64 + nb, j * 128 : (j + 1) * 128],
                            idento[64 : 64 + nb],
                        )
                # h^T = o'^T * tc^T written into the stationary tile; both
                # emitted after both halves' chains so neither blocks the
                # in-order DVE queue
                for n2 in range(2):
                    nc.vector.tensor_mul(
                        ht_new[:, 3 * n2 * nb : (3 * n2 + 3) * nb],
                        oT[n2], tcT_ps[n2],
                    )
                for n2 in range(2):
                    nc.sync.dma_start(
                        out=cs_d[:, t, n2], in_=C[n2][32 : 32 + nb]
                    )
                    c_prev[n2] = C[n2]
                nc.sync.dma_start(out=hs_d[t, :, :], in_=ht_new)

                ht_prev = ht_new
                gates_cur = gates_next
                x_tile = x_next

    nc.finalize()
    return nc


# Column permutation: reference gate order (i, f, g~, o) -> kernel (i, f, o, g~)
def _gate_perm():
    return np.concatenate(
        [np.arange(0, H), np.arange(H, 2 * H), np.arange(3 * H, 4 * H),
         np.arange(2 * H, 3 * H)]
    )


def _prep_core_inputs(input_, h0, c0, Wp, Up, bp, t_steps):
    nb = input_.shape[0]
    xT = np.ascontiguousarray(
        input_[:, :t_steps].transpose(1, 2, 0).reshape(t_steps, KX, 128, nb)
    )
    return {
        "xT": xT,
        "h0": np.ascontiguousarray(h0),
        "c0": np.ascontiguousarray(c0.reshape(nb, 2, NHALF).astype(np.float16)),
        "w": Wp,
        "u": Up,
        "b": bp,
    }


def run(input, hiddenState, cellState, W, U, b, t_steps=T, trace=False):
    input = np.asarray(input, np.float32)
    hiddenState = np.asarray(hiddenState, np.float32)
    cellState = np.asarray(cellState, np.float32)
    W = np.asarray(W, np.float32)
    U = np.asarray(U, np.float32)
    b = np.asarray(b, np.float32)

    perm = _gate_perm()
    Wp = np.ascontiguousarray(W[:, perm].reshape(KX, 128, 4 * H))
    Up = np.ascontiguousarray(U[:, perm].reshape(KH, 128, 4 * H))
    bp = np.ascontiguousarray(b[perm].reshape(1, 4 * H))
    has_b = bool(np.any(b))

    nc = build_lstm(NB, t_steps, has_b)
    in_maps = []
    for c in range(NCORES):
        bs = slice(c * NB, (c + 1) * NB)
        in_maps.append(
            _prep_core_inputs(
                input[bs], hiddenState[bs], cellState[bs], Wp, Up, bp, t_steps
            )
        )
    res = run_bass_kernel_spmd(
        nc, in_maps, core_ids=list(range(NCORES)), trace=trace
    )

    hs = np.empty((B, t_steps, H), np.float32)
    cs = np.empty((B, t_steps, H), np.float32)
    for c in range(NCORES):
        bs = slice(c * NB, (c + 1) * NB)
        ht = res.results[c]["hs"].astype(np.float32)  # [t, 128, 6*16]
        ht = ht.reshape(t_steps, 128, KH, NB)
        hs[bs] = ht.transpose(3, 0, 2, 1).reshape(NB, t_steps, H)
        cs[bs] = res.results[c]["cs"].astype(np.float32).reshape(NB, t_steps, H)
    return (hs, cs), res


def kernel(input, hiddenState, cellState, W, U, b):
    (hs, cs), _ = run(input, hiddenState, cellState, W, U, b)
    return hs, cs


# revision 12
# speedup vs baseline: 1.3076x; 1.0003x over previous
"""LSTM (BaseRNN) Trainium2 kernel.

Problem: B=128, T=512, I=256, H=768 LSTM; returns (hiddenStates, cellStates)
each [B, T, H] fp32.

Strategy (data-parallel over batch, 8 cores x 16 rows):
  - Batch-major gate preactivations g_t = x_t W + h_{t-1} U accumulated in
    PSUM as [batch, gate_cols]; stationary operand = x^T / h^T chunks
    [128, 16] fp16, moving operand = W/U chunks [128, 384] fp16.
  - 4-way PE col-group tiling: gate q -> col group q (psum partitions
    32q..); every group's first matmul carries start=True.
  - Host permutes gate columns to (i, f, o, g~).  One ACT sigmoid with a
    per-partition scale vector (1 for i/f/o rows, 2 for g~ rows) covers all
    four gates; tanh(x) = 2*sigmoid(2x)-1 is reconstructed by a cheap DVE
    tensor_scalar.
  - The step is processed in two independent column halves (psum banks);
    gate values, c, and h are fp16.
  - Pipelining for latency (the recurrent cycle is the bound):
      * next step's x-waves are emitted right after this step's h-waves so
        the PE stays busy (HAM stays warm) during the elementwise phase;
      * h-waves are ordered [h0-chunks psum0, h0-chunks psum1, h1-chunks
        psum0, h1-chunks psum1] so each half's sigmoid fires as early as
        possible;
      * G is written into spare partitions of the c_prev tile so f'*c and
        i'*g~ fuse into one [48,384] DVE multiply;
      * oT psum->sbuf copies run on DVE (ACT does only sigmoid+tanh).
  - h_t half is re-transposed via 3 PE-transposes into the fp16 h^T
    stationary for the next step.
  - hs/cs stream to DRAM as fp16; the host upcasts to fp32.
"""

import numpy as np

import concourse.bass as bass
import concourse.bacc as bacc
import concourse.tile as tile
from concourse import mybir
from concourse.bass_utils import run_bass_kernel_spmd
from concourse.masks import make_identity

B, T, I, H = 128, 512, 256, 768
NCORES = 8
NB = B // NCORES  # 16
KX = I // 128  # 2 x chunks
KH = H // 128  # 6 h chunks
NK = KX + KH  # 8 contraction waves
NHALF = H // 2  # 384: per-gate psum half (one matmul's N)
F32 = mybir.dt.float32
F16 = mybir.dt.float16
XBLK = 32  # x-stream block (steps per DMA)

MM_DT = F16  # matmul operand dtype


def build_lstm(nb=NB, t_steps=T, has_b=False):
    nc = bacc.Bacc(None, target_bir_lowering=False)

    xT_d = nc.dram_tensor("xT", [t_steps, KX, 128, nb], F32, kind="ExternalInput")
    h0_d = nc.dram_tensor("h0", [nb, H], F32, kind="ExternalInput")
    c0_d = nc.dram_tensor("c0", [nb, 2, NHALF], F16, kind="ExternalInput")
    w_d = nc.dram_tensor("w", [KX, 128, 4 * H], F32, kind="ExternalInput")
    u_d = nc.dram_tensor("u", [KH, 128, 4 * H], F32, kind="ExternalInput")
    b_d = nc.dram_tensor("b", [1, 4 * H], F32, kind="ExternalInput")
    hs_d = nc.dram_tensor("hs", [t_steps, 128, KH * NB], F16, kind="ExternalOutput")
    cs_d = nc.dram_tensor("cs", [nb, t_steps, 2, NHALF], F16, kind="ExternalOutput")

    SIG = mybir.ActivationFunctionType.Sigmoid
    TANH = mybir.ActivationFunctionType.Tanh
    MULT = mybir.AluOpType.mult
    ADD = mybir.AluOpType.add

    with tile.TileContext(nc) as tc:
        with (
            tc.tile_pool(name="consts", bufs=1) as consts,
            tc.tile_pool(name="xs", bufs=2) as xs_pool,
            tc.tile_pool(name="gsb", bufs=3) as gsb,
            tc.tile_pool(name="ew", bufs=3) as ew,
            tc.tile_pool(name="state", bufs=2) as state,
            tc.tile_pool(name="pg", bufs=2, space="PSUM") as pg,
            tc.tile_pool(name="pt", bufs=2, space="PSUM") as pt,
        ):
            ident16 = consts.tile([nb, nb], F16)
            make_identity(nc, ident16)
            idento = consts.tile([64 + nb, nb], F16)
            make_identity(nc, idento[64 : 64 + nb])
            # per-partition activation input scale: 1 for i/f/o rows,
            # 2 for g~ rows (tanh(x) = 2*sigmoid(2x) - 1)
            sc = consts.tile([112, 1], F32)
            nc.vector.memset(sc[0:96], 1.0)
            nc.vector.memset(sc[96:112], 2.0)

            # weights: DMA f32 staging -> round-convert to fp16
            w_sb = consts.tile([128, KX, 4 * H], MM_DT)
            u_sb = consts.tile([128, KH, 4 * H], MM_DT)
            for k in range(KX):
                stg = xs_pool.tile([128, 4 * H], F32, tag="WSTG")
                nc.sync.dma_start(out=stg, in_=w_d[k, :, :])
                nc.vector.tensor_copy(w_sb[:, k, :], stg)
            for k in range(KH):
                stg = xs_pool.tile([128, 4 * H], F32, tag="WSTG")
                nc.sync.dma_start(out=stg, in_=u_d[k, :, :])
                nc.vector.tensor_copy(u_sb[:, k, :], stg)
            if has_b:
                b_sb = consts.tile([1, 4 * H], F32)
                nc.sync.dma_start(out=b_sb, in_=b_d[:, :])

            # ---- initial state ----
            h0_sb = consts.tile([nb, H], F32)
            nc.sync.dma_start(out=h0_sb, in_=h0_d[:, :])
            c_prev = [None, None]
            for n2 in range(2):
                ct = state.tile([32 + nb, NHALF], F16, tag=f"C{n2}", bufs=3)
                nc.sync.dma_start(out=ct[32 : 32 + nb], in_=c0_d[:, n2, :])
                c_prev[n2] = ct

            h0_16 = consts.tile([nb, H], F16)
            nc.vector.tensor_copy(h0_16, h0_sb)
            ht_prev = state.tile([128, KH * nb], MM_DT, tag="HT", bufs=3)
            for n2 in range(2):
                ht0_ps = pt.tile([128, 3 * nb], F16, tag="tcTps")
                for j in range(3):
                    ck = 3 * n2 + j
                    nc.tensor.transpose(
                        ht0_ps[:, j * nb : (j + 1) * nb],
                        h0_16[:, ck * 128 : (ck + 1) * 128],
                        ident16,
                    )
                nc.scalar.copy(
                    out=ht_prev[:, 3 * n2 * nb : (3 * n2 + 3) * nb], in_=ht0_ps
                )

            def load_xblk(t0):
                """DMA + f16-convert the x block starting at step t0."""
                xstg = xs_pool.tile([128, XBLK, KX, nb], F32, tag="XSTG")
                nblk = min(XBLK, t_steps - t0)
                nc.sync.dma_start(
                    out=xstg[:, 0:nblk],
                    in_=xT_d[t0 : t0 + nblk].rearrange("t k p b -> p t k b"),
                )
                xt = xs_pool.tile([128, XBLK, KX, nb], MM_DT, tag="X")
                nc.vector.tensor_copy(xt[:, 0:nblk], xstg[:, 0:nblk])
                return xt

            def x_waves(gates_tile, xt, trel, add_bias):
                """Emit the x-projection waves (accumulation starters)."""
                for n2 in range(2):
                    for k in range(KX):
                        for q in range(4):
                            col = q * H + n2 * NHALF
                            nc.tensor.matmul(
                                gates_tile[32 * q : 32 * q + nb, n2, 0:NHALF],
                                xt[:, trel, k, :],
                                w_sb[:, k, col : col + NHALF],
                                start=(k == 0),
                                stop=False,
                                tile_position=(0, 32 * q),
                                skip_group_check=True,
                            )
                if add_bias:
                    for n2 in range(2):
                        for q, base in ((0, 0), (1, 32), (2, 64), (3, 96)):
                            bq = b_sb[:, q * H + n2 * NHALF : q * H + (n2 + 1) * NHALF]
                            bq = bass.AP(
                                tensor=bq.tensor, offset=bq.offset,
                                ap=[[0, nb]] + bq.ap[1:],
                            )
                            nc.vector.tensor_add(
                                gates_tile[base : base + nb, n2, 0:NHALF],
                                gates_tile[base : base + nb, n2, 0:NHALF],
                                bq,
                            )

            # x block 0 + step-0 x-waves
            x_tile = load_xblk(0)
            gates_cur = pg.tile([128, 2, 512], F32, tag="gates")
            # bias path handled via DVE adds after accumulation finishes, so
            # hoisted x-waves stay start-only; adds are emitted with h-waves.
            x_waves(gates_cur, x_tile, 0, False)

            for t in range(t_steps):
                trel1 = (t + 1) % XBLK
                x_next = x_tile
                if t + 1 < t_steps and trel1 == 0:
                    x_next = load_xblk(t + 1)

                # --- h-waves for step t: h0-dependent chunks (0-2) for both
                # psum halves first, then h1-dependent chunks (3-5) ---
                for ckgrp in range(2):
                    for n2 in range(2):
                        for kc in range(3):
                            ck = 3 * ckgrp + kc
                            for q in range(4):
                                col = q * H + n2 * NHALF
                                nc.tensor.matmul(
                                    gates_cur[32 * q : 32 * q + nb, n2, 0:NHALF],
                                    ht_prev[:, ck * nb : (ck + 1) * nb],
                                    u_sb[:, ck, col : col + NHALF],
                                    start=False,
                                    stop=(ck == KH - 1),
                                    tile_position=(0, 32 * q),
                                    skip_group_check=True,
                                )

                if has_b:
                    for n2 in range(2):
                        for q, base in ((0, 0), (1, 32), (2, 64), (3, 96)):
                            bq = b_sb[:, q * H + n2 * NHALF : q * H + (n2 + 1) * NHALF]
                            bq = bass.AP(
                                tensor=bq.tensor, offset=bq.offset,
                                ap=[[0, nb]] + bq.ap[1:],
                            )
                            nc.vector.tensor_add(
                                gates_cur[base : base + nb, n2, 0:NHALF],
                                gates_cur[base : base + nb, n2, 0:NHALF],
                                bq,
                            )

                # --- next step's x-waves: keep the PE busy during the
                # elementwise phase (HAM stays warm) ---
                gates_next = None
                if t + 1 < t_steps:
                    gates_next = pg.tile([128, 2, 512], F32, tag="gates")
                    x_waves(gates_next, x_next, trel1, False)

                def dummy_mms(n):
                    # Warm-keeper matmuls into the unused 384:512 column
                    # region of the next gate psum tile: the PE's HAM clock
                    # gate re-throttles to 1.2 GHz whenever the PE idles for
                    # part of a 4096-cycle window, which would make every
                    # wave ~2x slower. These fillers run during the
                    # elementwise-phase dependency stalls.
                    if gates_next is None:
                        return
                    for i in range(n):
                        nc.tensor.matmul(
                            gates_next[0:nb, i % 2, 384:512],
                            ht_prev[:, 0:nb],
                            u_sb[:, 0, 0:128],
                            start=False,
                            stop=False,
                            tile_position=(0, 0),
                            skip_group_check=True,
                        )

                dummy_mms(6)

                # --- elementwise, half-major: half0's whole DVE chain is
                # emitted (and runs) before half1's, so half0's tail
                # (tanh/transpose/h^T) overlaps half1's DVE work and the
                # next step's h0-chunk waves start as early as possible ---
                # S2 rows: i'@0:16, f'@32:48, o'@64:80, sigmoid(2g)@96:112
                S2, C, G, T1, TC = {}, {}, {}, {}, {}
                for n2 in range(2):
                    S2[n2] = gsb.tile([112, NHALF], F16, tag=f"S2{n2}", name=f"S2_{n2}")
                    nc.scalar.activation(
                        out=S2[n2], in_=gates_cur[0:112, n2, 0:NHALF],
                        func=SIG, scale=sc,
                    )
                oT_ps, oT, tcT_ps = {}, {}, {}
                ht_new = state.tile([128, KH * nb], MM_DT, tag="HT", bufs=3)
                for n2 in range(2):
                    # g~ = 2*sigmoid(2g) - 1
                    G[n2] = gsb.tile([nb, NHALF], F16, tag=f"G{n2}", name=f"G_{n2}")
                    nc.vector.tensor_scalar(
                        G[n2], S2[n2][96:112], 2.0, -1.0, MULT, ADD
                    )
                    # f' * c  (into C rows 32:48)
                    C[n2] = state.tile([32 + nb, NHALF], F16, tag=f"C{n2}", name=f"C_{n2}", bufs=3)
                    nc.vector.tensor_mul(
                        C[n2][32 : 32 + nb], S2[n2][32 : 32 + nb],
                        c_prev[n2][32 : 32 + nb],
                    )
                    # i' * g~ (out-shift to rows 32:48)
                    T1[n2] = ew.tile([32 + nb, NHALF], F16, tag=f"T1{n2}", name=f"T1_{n2}")
                    nc.vector.tensor_mul(
                        T1[n2][32 : 32 + nb], S2[n2][0:nb], G[n2]
                    )
                    nc.vector.tensor_add(
                        C[n2][32 : 32 + nb], C[n2][32 : 32 + nb],
                        T1[n2][32 : 32 + nb],
                    )
                    # o'^T into psum (PE), then ACT copies it to SBUF (ACT
                    # has slack during the DVE chain; keeps DVE queue clear)
                    oT_ps[n2] = pt.tile([128, 3 * nb], F16, tag="oTps", name=f"oTps_{n2}")
                    for j in range(3):
                        nc.tensor.transpose(
                            oT_ps[n2][:, j * nb : (j + 1) * nb],
                            S2[n2][64 : 64 + nb, j * 128 : (j + 1) * 128],
                            idento[64 : 64 + nb],
                        )
                    oT[n2] = ew.tile([128, 3 * nb], F16, tag=f"oT{n2}", name=f"oT_{n2}")
                    nc.scalar.copy(out=oT[n2], in_=oT_ps[n2])
                    TC[n2] = ew.tile([64 + nb, NHALF], F16, tag=f"TC{n2}", name=f"TC_{n2}")
                    nc.scalar.activation(
                        out=TC[n2][64 : 64 + nb], in_=C[n2][32 : 32 + nb],
                        func=TANH,
                    )
                    if n2 == 0:
                        dummy_mms(6)
                    # tc^T via PE transpose (stays in psum)
                    tcT_ps[n2] = pt.tile([128, 3 * nb], F16, tag="tcTps", name=f"tcTps_{n2}")
                    for j in range(3):
                        nc.tensor.transpose(
                            tcT_ps[n2][:, j * nb : (j + 1) * nb],
                            TC[n2][64 : 64 + nb, j * 128 : (j + 1) * 128],
                            idento[64 : 64 + nb],
                        )
                # h^T = o'^T * tc^T written into the stationary tile; both
                # emitted after both halves' chains so neither blocks the
                # in-order DVE queue
                for n2 in range(2):
                    nc.vector.tensor_mul(
                        ht_new[:, 3 * n2 * nb : (3 * n2 + 3) * nb],
                        oT[n2], tcT_ps[n2],
                    )
                for n2 in range(2):
                    nc.sync.dma_start(
                        out=cs_d[:, t, n2], in_=C[n2][32 : 32 + nb]
                    )
                    c_prev[n2] = C[n2]
                nc.sync.dma_start(out=hs_d[t, :, :], in_=ht_new)

                ht_prev = ht_new
                gates_cur = gates_next
                x_tile = x_next

    nc.finalize()
    return nc


# Column permutation: reference gate order (i, f, g~, o) -> kernel (i, f, o, g~)
def _gate_perm():
    return np.concatenate(
        [np.arange(0, H), np.arange(H, 2 * H), np.arange(3 * H, 4 * H),
         np.arange(2 * H, 3 * H)]
    )


def _prep_core_inputs(input_, h0, c0, Wp, Up, bp, t_steps):
    nb = input_.shape[0]
    xT = np.ascontiguousarray(
        input_[:, :t_steps].transpose(1, 2, 0).reshape(t_steps, KX, 128, nb)
    )
    return {
        "xT": xT,
        "h0": np.ascontiguousarray(h0),
        "c0": np.ascontiguousarray(c0.reshape(nb, 2, NHALF).astype(np.float16)),
        "w": Wp,
        "u": Up,
        "b": bp,
    }


def run(input, hiddenState, cellState, W, U, b, t_steps=T, trace=False):
    input = np.asarray(input, np.float32)
    hiddenState = np.asarray(hiddenState, np.float32)
    cellState = np.asarray(cellState, np.float32)
    W = np.asarray(W, np.float32)
    U = np.asarray(U, np.float32)
    b = np.asarray(b, np.float32)

    perm = _gate_perm()
    Wp = np.ascontiguousarray(W[:, perm].reshape(KX, 128, 4 * H))
    Up = np.ascontiguousarray(U[:, perm].reshape(KH, 128, 4 * H))
    bp = np.ascontiguousarray(b[perm].reshape(1, 4 * H))
    has_b = bool(np.any(b))

    nc = build_lstm(NB, t_steps, has_b)
    in_maps = []
    for c in range(NCORES):
        bs = slice(c * NB, (c + 1) * NB)
        in_maps.append(
            _prep_core_inputs(
                input[bs], hiddenState[bs], cellState[bs], Wp, Up, bp, t_steps
            )
        )
    res = run_bass_kernel_spmd(
        nc, in_maps, core_ids=list(range(NCORES)), trace=trace
    )

    hs = np.empty((B, t_steps, H), np.float32)
    cs = np.empty((B, t_steps, H), np.float32)
    for c in range(NCORES):
        bs = slice(c * NB, (c + 1) * NB)
        ht = res.results[c]["hs"].astype(np.float32)  # [t, 128, 6*16]
        ht = ht.reshape(t_steps, 128, KH, NB)
        hs[bs] = ht.transpose(3, 0, 2, 1).reshape(NB, t_steps, H)
        cs[bs] = res.results[c]["cs"].astype(np.float32).reshape(NB, t_steps, H)
    return (hs, cs), res


def kernel(input, hiddenState, cellState, W, U, b):
    (hs, cs), _ = run(input, hiddenState, cellState, W, U, b)
    return hs, cs


# revision 15
# speedup vs baseline: 1.3102x; 1.0020x over previous
"""LSTM (BaseRNN) Trainium2 kernel.

Problem: B=128, T=512, I=256, H=768 LSTM; returns (hiddenStates, cellStates)
each [B, T, H] fp32.

Strategy (data-parallel over batch, 8 cores x 16 rows):
  - Batch-major gate preactivations g_t = x_t W + h_{t-1} U accumulated in
    PSUM as [batch, gate_cols]; stationary operand = x^T / h^T chunks
    [128, 16] fp16, moving operand = W/U chunks [128, 384] fp16.
  - 4-way PE col-group tiling: gate q -> col group q (psum partitions
    32q..); every group's first matmul carries start=True.
  - Host permutes gate columns to (i, f, o, g~).  One ACT sigmoid with a
    per-partition scale vector (1 for i/f/o rows, 2 for g~ rows) covers all
    four gates; tanh(x) = 2*sigmoid(2x)-1 is reconstructed by a cheap DVE
    tensor_scalar.
  - The step is processed in two independent column halves (psum banks);
    gate values, c, and h are fp16.
  - Pipelining for latency (the recurrent cycle is the bound):
      * next step's x-waves are emitted right after this step's h-waves so
        the PE stays busy (HAM stays warm) during the elementwise phase;
      * h-waves are ordered [h0-chunks psum0, h0-chunks psum1, h1-chunks
        psum0, h1-chunks psum1] so each half's sigmoid fires as early as
        possible;
      * G is written into spare partitions of the c_prev tile so f'*c and
        i'*g~ fuse into one [48,384] DVE multiply;
      * oT psum->sbuf copies run on DVE (ACT does only sigmoid+tanh).
  - h_t half is re-transposed via 3 PE-transposes into the fp16 h^T
    stationary for the next step.
  - hs/cs stream to DRAM as fp16; the host upcasts to fp32.
"""

import numpy as np

import concourse.bass as bass
import concourse.bacc as bacc
import concourse.tile as tile
from concourse import mybir
from concourse.bass_utils import run_bass_kernel_spmd
from concourse.masks import make_identity

B, T, I, H = 128, 512, 256, 768
NCORES = 8
NB = B // NCORES  # 16
KX = I // 128  # 2 x chunks
KH = H // 128  # 6 h chunks
NK = KX + KH  # 8 contraction waves
NHALF = H // 2  # 384: per-gate psum half (one matmul's N)
F32 = mybir.dt.float32
F16 = mybir.dt.float16
XBLK = 32  # x-stream block (steps per DMA)

MM_DT = F16  # matmul operand dtype


def build_lstm(nb=NB, t_steps=T, has_b=False):
    nc = bacc.Bacc(None, target_bir_lowering=False)

    xT_d = nc.dram_tensor("xT", [t_steps, KX, 128, nb], F32, kind="ExternalInput")
    h0_d = nc.dram_tensor("h0", [nb, H], F32, kind="ExternalInput")
    c0_d = nc.dram_tensor("c0", [nb, 2, NHALF], F16, kind="ExternalInput")
    w_d = nc.dram_tensor("w", [KX, 128, 4 * H], F32, kind="ExternalInput")
    u_d = nc.dram_tensor("u", [KH, 128, 4 * H], F32, kind="ExternalInput")
    b_d = nc.dram_tensor("b", [1, 4 * H], F32, kind="ExternalInput")
    hs_d = nc.dram_tensor("hs", [t_steps, 128, KH * NB], F16, kind="ExternalOutput")
    cs_d = nc.dram_tensor("cs", [nb, t_steps, 2, NHALF], F16, kind="ExternalOutput")

    SIG = mybir.ActivationFunctionType.Sigmoid
    TANH = mybir.ActivationFunctionType.Tanh
    MULT = mybir.AluOpType.mult
    ADD = mybir.AluOpType.add

    with tile.TileContext(nc) as tc:
        with (
            tc.tile_pool(name="consts", bufs=1) as consts,
            tc.tile_pool(name="xs", bufs=2) as xs_pool,
            tc.tile_pool(name="gsb", bufs=3) as gsb,
            tc.tile_pool(name="ew", bufs=3) as ew,
            tc.tile_pool(name="state", bufs=2) as state,
            tc.tile_pool(name="pg", bufs=2, space="PSUM") as pg,
            tc.tile_pool(name="pt", bufs=2, space="PSUM") as pt,
        ):
            ident16 = consts.tile([nb, nb], F16)
            make_identity(nc, ident16)
            idento = consts.tile([64 + nb, nb], F16)
            make_identity(nc, idento[64 : 64 + nb])
            # per-partition activation input scale: 1 for i/f/o rows,
            # 2 for g~ rows (tanh(x) = 2*sigmoid(2x) - 1)
            sc = consts.tile([112, 1], F32)
            nc.vector.memset(sc[0:96], 1.0)
            nc.vector.memset(sc[96:112], 2.0)

            # weights: DMA f32 staging -> round-convert to fp16
            w_sb = consts.tile([128, KX, 4 * H], MM_DT)
            u_sb = consts.tile([128, KH, 4 * H], MM_DT)
            for k in range(KX):
                stg = xs_pool.tile([128, 4 * H], F32, tag="WSTG")
                nc.sync.dma_start(out=stg, in_=w_d[k, :, :])
                nc.vector.tensor_copy(w_sb[:, k, :], stg)
            for k in range(KH):
                stg = xs_pool.tile([128, 4 * H], F32, tag="WSTG")
                nc.sync.dma_start(out=stg, in_=u_d[k, :, :])
                nc.vector.tensor_copy(u_sb[:, k, :], stg)
            if has_b:
                b_sb = consts.tile([1, 4 * H], F32)
                nc.sync.dma_start(out=b_sb, in_=b_d[:, :])

            # ---- initial state ----
            h0_sb = consts.tile([nb, H], F32)
            nc.sync.dma_start(out=h0_sb, in_=h0_d[:, :])
            c_prev = [None, None]
            for n2 in range(2):
                ct = state.tile([32 + nb, NHALF], F16, tag=f"C{n2}", bufs=3)
                nc.sync.dma_start(out=ct[32 : 32 + nb], in_=c0_d[:, n2, :])
                c_prev[n2] = ct

            h0_16 = consts.tile([nb, H], F16)
            nc.vector.tensor_copy(h0_16, h0_sb)
            ht_prev = state.tile([128, KH * nb], MM_DT, tag="HT", bufs=3)
            for n2 in range(2):
                ht0_ps = pt.tile([128, 3 * nb], F16, tag="tcTps")
                for j in range(3):
                    ck = 3 * n2 + j
                    nc.tensor.transpose(
                        ht0_ps[:, j * nb : (j + 1) * nb],
                        h0_16[:, ck * 128 : (ck + 1) * 128],
                        ident16,
                    )
                nc.scalar.copy(
                    out=ht_prev[:, 3 * n2 * nb : (3 * n2 + 3) * nb], in_=ht0_ps
                )

            def load_xblk(t0):
                """DMA + f16-convert the x block starting at step t0."""
                xstg = xs_pool.tile([128, XBLK, KX, nb], F32, tag="XSTG", bufs=3)
                nblk = min(XBLK, t_steps - t0)
                nc.sync.dma_start(
                    out=xstg[:, 0:nblk],
                    in_=xT_d[t0 : t0 + nblk].rearrange("t k p b -> p t k b"),
                )
                xt = xs_pool.tile([128, XBLK, KX, nb], MM_DT, tag="X", bufs=3)
                nc.vector.tensor_copy(xt[:, 0:nblk], xstg[:, 0:nblk])
                return xt

            def x_waves(gates_tile, xt, trel, add_bias):
                """Emit the x-projection waves (accumulation starters)."""
                for n2 in range(2):
                    for k in range(KX):
                        for q in range(4):
                            col = q * H + n2 * NHALF
                            nc.tensor.matmul(
                                gates_tile[32 * q : 32 * q + nb, n2, 0:NHALF],
                                xt[:, trel, k, :],
                                w_sb[:, k, col : col + NHALF],
                                start=(k == 0),
                                stop=False,
                                tile_position=(0, 32 * q),
                                skip_group_check=True,
                            )
                if add_bias:
                    for n2 in range(2):
                        for q, base in ((0, 0), (1, 32), (2, 64), (3, 96)):
                            bq = b_sb[:, q * H + n2 * NHALF : q * H + (n2 + 1) * NHALF]
                            bq = bass.AP(
                                tensor=bq.tensor, offset=bq.offset,
                                ap=[[0, nb]] + bq.ap[1:],
                            )
                            nc.vector.tensor_add(
                                gates_tile[base : base + nb, n2, 0:NHALF],
                                gates_tile[base : base + nb, n2, 0:NHALF],
                                bq,
                            )

            # x block 0 + step-0 x-waves
            xtiles = {0: load_xblk(0)}
            if t_steps > XBLK:
                xtiles[1] = load_xblk(XBLK)
            gates_cur = pg.tile([128, 2, 512], F32, tag="gates")
            # bias path handled via DVE adds after accumulation finishes, so
            # hoisted x-waves stay start-only; adds are emitted with h-waves.
            x_waves(gates_cur, xtiles[0], 0, False)

            for t in range(t_steps):
                trel1 = (t + 1) % XBLK
                # prefetch the x block needed ~XBLK steps from now so its
                # DMA + f16 convert never lands on the critical path
                tpre = t + 4
                if tpre % XBLK == 0 and tpre + XBLK < t_steps:
                    bi = tpre // XBLK + 1
                    xtiles[bi] = load_xblk(bi * XBLK)
                x_next = xtiles[(t + 1) // XBLK] if t + 1 < t_steps else None

                # --- h-waves for step t: h0-dependent chunks (0-2) for both
                # psum halves first, then h1-dependent chunks (3-5) ---
                for ckgrp in range(2):
                    for n2 in range(2):
                        for kc in range(3):
                            ck = 3 * ckgrp + kc
                            for q in range(4):
                                col = q * H + n2 * NHALF
                                nc.tensor.matmul(
                                    gates_cur[32 * q : 32 * q + nb, n2, 0:NHALF],
                                    ht_prev[:, ck * nb : (ck + 1) * nb],
                                    u_sb[:, ck, col : col + NHALF],
                                    start=False,
                                    stop=(ck == KH - 1),
                                    tile_position=(0, 32 * q),
                                    skip_group_check=True,
                                )

                if has_b:
                    for n2 in range(2):
                        for q, base in ((0, 0), (1, 32), (2, 64), (3, 96)):
                            bq = b_sb[:, q * H + n2 * NHALF : q * H + (n2 + 1) * NHALF]
                            bq = bass.AP(
                                tensor=bq.tensor, offset=bq.offset,
                                ap=[[0, nb]] + bq.ap[1:],
                            )
                            nc.vector.tensor_add(
                                gates_cur[base : base + nb, n2, 0:NHALF],
                                gates_cur[base : base + nb, n2, 0:NHALF],
                                bq,
                            )

                # --- next step's x-waves: keep the PE busy during the
                # elementwise phase (HAM stays warm) ---
                gates_next = None
                if t + 1 < t_steps:
                    gates_next = pg.tile([128, 2, 512], F32, tag="gates")
                    x_waves(gates_next, x_next, trel1, False)

                def dummy_mms(n):
                    # Warm-keeper matmuls into the unused 384:512 column
                    # region of the next gate psum tile: the PE's HAM clock
                    # gate re-throttles to 1.2 GHz whenever the PE idles for
                    # part of a 4096-cycle window, which would make every
                    # wave ~2x slower. These fillers run during the
                    # elementwise-phase dependency stalls.
                    if gates_next is None:
                        return
                    for i in range(n):
                        nc.tensor.matmul(
                            gates_next[0:nb, i % 2, 384:512],
                            ht_prev[:, 0:nb],
                            u_sb[:, 0, 0:128],
                            start=False,
                            stop=False,
                            tile_position=(0, 0),
                            skip_group_check=True,
                        )

                dummy_mms(6)

                # --- elementwise, half-major: half0's whole DVE chain is
                # emitted (and runs) before half1's, so half0's tail
                # (tanh/transpose/h^T) overlaps half1's DVE work and the
                # next step's h0-chunk waves start as early as possible ---
                # S2 rows: i'@0:16, f'@32:48, o'@64:80, sigmoid(2g)@96:112
                S2, C, G, T1, TC = {}, {}, {}, {}, {}
                for n2 in range(2):
                    S2[n2] = gsb.tile([112, NHALF], F16, tag=f"S2{n2}", name=f"S2_{n2}")
                    nc.scalar.activation(
                        out=S2[n2], in_=gates_cur[0:112, n2, 0:NHALF],
                        func=SIG, scale=sc,
                    )
                oT_ps, oT, tcT_ps = {}, {}, {}
                ht_new = state.tile([128, KH * nb], MM_DT, tag="HT", bufs=3)
                for n2 in range(2):
                    # g~ = 2*sigmoid(2g) - 1
                    G[n2] = gsb.tile([nb, NHALF], F16, tag=f"G{n2}", name=f"G_{n2}")
                    nc.vector.tensor_scalar(
                        G[n2], S2[n2][96:112], 2.0, -1.0, MULT, ADD
                    )
                    # f' * c  (into C rows 32:48)
                    C[n2] = state.tile([32 + nb, NHALF], F16, tag=f"C{n2}", name=f"C_{n2}", bufs=3)
                    nc.vector.tensor_mul(
                        C[n2][32 : 32 + nb], S2[n2][32 : 32 + nb],
                        c_prev[n2][32 : 32 + nb],
                    )
                    # i' * g~ (out-shift to rows 32:48)
                    T1[n2] = ew.tile([32 + nb, NHALF], F16, tag=f"T1{n2}", name=f"T1_{n2}")
                    nc.vector.tensor_mul(
                        T1[n2][32 : 32 + nb], S2[n2][0:nb], G[n2]
                    )
                    nc.vector.tensor_add(
                        C[n2][32 : 32 + nb], C[n2][32 : 32 + nb],
                        T1[n2][32 : 32 + nb],
                    )
                    # o'^T into psum (PE), then ACT copies it to SBUF (ACT
                    # has slack during the DVE chain; keeps DVE queue clear)
                    oT_ps[n2] = pt.tile([128, 3 * nb], F16, tag="oTps", name=f"oTps_{n2}")
                    for j in range(3):
                        nc.tensor.transpose(
                            oT_ps[n2][:, j * nb : (j + 1) * nb],
                            S2[n2][64 : 64 + nb, j * 128 : (j + 1) * 128],
                            idento[64 : 64 + nb],
                        )
                    oT[n2] = ew.tile([128, 3 * nb], F16, tag=f"oT{n2}", name=f"oT_{n2}")
                    nc.scalar.copy(out=oT[n2], in_=oT_ps[n2])
                    TC[n2] = ew.tile([64 + nb, NHALF], F16, tag=f"TC{n2}", name=f"TC_{n2}")
                    nc.scalar.activation(
                        out=TC[n2][64 : 64 + nb], in_=C[n2][32 : 32 + nb],
                        func=TANH,
                    )
                    if n2 == 0:
                        dummy_mms(6)
                    # tc^T via PE transpose (stays in psum)
                    tcT_ps[n2] = pt.tile([128, 3 * nb], F16, tag="tcTps", name=f"tcTps_{n2}")
                    for j in range(3):
                        nc.tensor.transpose(
                            tcT_ps[n2][:, j * nb : (j + 1) * nb],
                            TC[n2][64 : 64 + nb, j * 128 : (j + 1) * 128],
                            idento[64 : 64 + nb],
                        )
                # h^T = o'^T * tc^T written into the stationary tile; both
                # emitted after both halves' chains so neither blocks the
                # in-order DVE queue
                for n2 in range(2):
                    nc.vector.tensor_mul(
                        ht_new[:, 3 * n2 * nb : (3 * n2 + 3) * nb],
                        oT[n2], tcT_ps[n2],
                    )
                for n2 in range(2):
                    nc.sync.dma_start(
                        out=cs_d[:, t, n2], in_=C[n2][32 : 32 + nb]
                    )
                    c_prev[n2] = C[n2]
                nc.sync.dma_start(out=hs_d[t, :, :], in_=ht_new)

                ht_prev = ht_new
                gates_cur = gates_next
                x_tile = x_next

    nc.finalize()
    return nc


# Column permutation: reference gate order (i, f, g~, o) -> kernel (i, f, o, g~)
def _gate_perm():
    return np.concatenate(
        [np.arange(0, H), np.arange(H, 2 * H), np.arange(3 * H, 4 * H),
         np.arange(2 * H, 3 * H)]
    )


def _prep_core_inputs(input_, h0, c0, Wp, Up, bp, t_steps):
    nb = input_.shape[0]
    xT = np.ascontiguousarray(
        input_[:, :t_steps].transpose(1, 2, 0).reshape(t_steps, KX, 128, nb)
    )
    return {
        "xT": xT,
        "h0": np.ascontiguousarray(h0),
        "c0": np.ascontiguousarray(c0.reshape(nb, 2, NHALF).astype(np.float16)),
        "w": Wp,
        "u": Up,
        "b": bp,
    }


def run(input, hiddenState, cellState, W, U, b, t_steps=T, trace=False):
    input = np.asarray(input, np.float32)
    hiddenState = np.asarray(hiddenState, np.float32)
    cellState = np.asarray(cellState, np.float32)
    W = np.asarray(W, np.float32)
    U = np.asarray(U, np.float32)
    b = np.asarray(b, np.float32)

    perm = _gate_perm()
    Wp = np.ascontiguousarray(W[:, perm].reshape(KX, 128, 4 * H))
    Up = np.ascontiguousarray(U[:, perm].reshape(KH, 128, 4 * H))
    bp = np.ascontiguousarray(b[perm].reshape(1, 4 * H))
    has_b = bool(np.any(b))

    nc = build_lstm(NB, t_steps, has_b)
    in_maps = []
    for c in range(NCORES):
        bs = slice(c * NB, (c + 1) * NB)
        in_maps.append(
            _prep_core_inputs(
                input[bs], hiddenState[bs], cellState[bs], Wp, Up, bp, t_steps
            )
        )
    res = run_bass_kernel_spmd(
        nc, in_maps, core_ids=list(range(NCORES)), trace=trace
    )

    hs = np.empty((B, t_steps, H), np.float32)
    cs = np.empty((B, t_steps, H), np.float32)
    for c in range(NCORES):
        bs = slice(c * NB, (c + 1) * NB)
        ht = res.results[c]["hs"].astype(np.float32)  # [t, 128, 6*16]
        ht = ht.reshape(t_steps, 128, KH, NB)
        hs[bs] = ht.transpose(3, 0, 2, 1).reshape(NB, t_steps, H)
        cs[bs] = res.results[c]["cs"].astype(np.float32).reshape(NB, t_steps, H)
    return (hs, cs), res


def kernel(input, hiddenState, cellState, W, U, b):
    (hs, cs), _ = run(input, hiddenState, cellState, W, U, b)
    return hs, cs


# revision 19
# speedup vs baseline: 1.3225x; 1.0093x over previous
"""LSTM (BaseRNN) Trainium2 kernel.

Problem: B=128, T=512, I=256, H=768 LSTM; returns (hiddenStates, cellStates)
each [B, T, H] fp32.

Strategy (data-parallel over batch, 8 cores x 16 rows):
  - Batch-major gate preactivations g_t = x_t W + h_{t-1} U accumulated in
    PSUM as [batch, gate_cols]; stationary operand = x^T / h^T chunks
    [128, 16] fp16, moving operand = W/U chunks [128, 384] fp16.
  - 4-way PE col-group tiling: gate q -> col group q (psum partitions
    32q..); every group's first matmul carries start=True.
  - Host permutes gate columns to (i, f, o, g~).  One ACT sigmoid with a
    per-partition scale vector (1 for i/f/o rows, 2 for g~ rows) covers all
    four gates; tanh(x) = 2*sigmoid(2x)-1 is reconstructed by a cheap DVE
    tensor_scalar.
  - The step is processed in two independent column halves (psum banks);
    gate values, c, and h are fp16.
  - Pipelining for latency (the recurrent cycle is the bound):
      * next step's x-waves are emitted right after this step's h-waves so
        the PE stays busy (HAM stays warm) during the elementwise phase;
      * h-waves are ordered [h0-chunks psum0, h0-chunks psum1, h1-chunks
        psum0, h1-chunks psum1] so each half's sigmoid fires as early as
        possible;
      * G is written into spare partitions of the c_prev tile so f'*c and
        i'*g~ fuse into one [48,384] DVE multiply;
      * oT psum->sbuf copies run on DVE (ACT does only sigmoid+tanh).
  - h_t half is re-transposed via 3 PE-transposes into the fp16 h^T
    stationary for the next step.
  - hs/cs stream to DRAM as fp16; the host upcasts to fp32.
"""

import numpy as np

import concourse.bass as bass
import concourse.bacc as bacc
import concourse.tile as tile
from concourse import mybir
from concourse.bass_utils import run_bass_kernel_spmd
from concourse.masks import make_identity

B, T, I, H = 128, 512, 256, 768
NCORES = 8
NB = B // NCORES  # 16
KX = I // 128  # 2 x chunks
KH = H // 128  # 6 h chunks
NK = KX + KH  # 8 contraction waves
NHALF = H // 2  # 384: per-gate psum half (one matmul's N)
F32 = mybir.dt.float32
F16 = mybir.dt.float16
XBLK = 32  # x-stream block (steps per DMA)

MM_DT = F16  # matmul operand dtype


def build_lstm(nb=NB, t_steps=T, has_b=False):
    nc = bacc.Bacc(None, target_bir_lowering=False)

    xT_d = nc.dram_tensor("xT", [t_steps, KX, 128, nb], F32, kind="ExternalInput")
    h0_d = nc.dram_tensor("h0", [nb, H], F32, kind="ExternalInput")
    c0_d = nc.dram_tensor("c0", [nb, 2, NHALF], F16, kind="ExternalInput")
    w_d = nc.dram_tensor("w", [KX, 128, 4 * H], F32, kind="ExternalInput")
    u_d = nc.dram_tensor("u", [KH, 128, 4 * H], F32, kind="ExternalInput")
    b_d = nc.dram_tensor("b", [1, 4 * H], F32, kind="ExternalInput")
    hs_d = nc.dram_tensor("hs", [t_steps, 128, KH * NB], F16, kind="ExternalOutput")
    cs_d = nc.dram_tensor("cs", [nb, t_steps, 2, NHALF], F16, kind="ExternalOutput")

    SIG = mybir.ActivationFunctionType.Sigmoid
    TANH = mybir.ActivationFunctionType.Tanh
    MULT = mybir.AluOpType.mult
    ADD = mybir.AluOpType.add

    with tile.TileContext(nc) as tc:
        with (
            tc.tile_pool(name="consts", bufs=1) as consts,
            tc.tile_pool(name="xs", bufs=2) as xs_pool,
            tc.tile_pool(name="gsb", bufs=3) as gsb,
            tc.tile_pool(name="ew", bufs=3) as ew,
            tc.tile_pool(name="state", bufs=2) as state,
            tc.tile_pool(name="pg", bufs=2, space="PSUM") as pg,
            tc.tile_pool(name="pt", bufs=2, space="PSUM") as pt,
        ):
            ident16 = consts.tile([nb, nb], F16)
            make_identity(nc, ident16)
            idento = consts.tile([64 + nb, nb], F16)
            make_identity(nc, idento[64 : 64 + nb])
            # per-partition activation input scale: 1 for i/f/o rows,
            # 2 for g~ rows (tanh(x) = 2*sigmoid(2x) - 1)
            sc = consts.tile([112, 1], F32)
            nc.vector.memset(sc[0:96], 1.0)
            nc.vector.memset(sc[96:112], 2.0)

            # weights: DMA f32 staging -> round-convert to fp16
            w_sb = consts.tile([128, KX, 4 * H], MM_DT)
            u_sb = consts.tile([128, KH, 4 * H], MM_DT)
            for k in range(KX):
                stg = xs_pool.tile([128, 4 * H], F32, tag="WSTG")
                nc.sync.dma_start(out=stg, in_=w_d[k, :, :])
                nc.vector.tensor_copy(w_sb[:, k, :], stg)
            for k in range(KH):
                stg = xs_pool.tile([128, 4 * H], F32, tag="WSTG")
                nc.sync.dma_start(out=stg, in_=u_d[k, :, :])
                nc.vector.tensor_copy(u_sb[:, k, :], stg)
            if has_b:
                b_sb = consts.tile([1, 4 * H], F32)
                nc.sync.dma_start(out=b_sb, in_=b_d[:, :])

            # ---- initial state ----
            h0_sb = consts.tile([nb, H], F32)
            nc.sync.dma_start(out=h0_sb, in_=h0_d[:, :])
            c_prev = state.tile([32 + nb, 2, NHALF], F16, tag="C", bufs=4)
            nc.sync.dma_start(out=c_prev[32 : 32 + nb], in_=c0_d[:, :, :])

            h0_16 = consts.tile([nb, H], F16)
            nc.vector.tensor_copy(h0_16, h0_sb)
            ht_prev = state.tile([128, KH * nb], MM_DT, tag="HT", bufs=4)
            for n2 in range(2):
                ht0_ps = pt.tile([128, 3 * nb], F16, tag="tcTps")
                for j in range(3):
                    ck = 3 * n2 + j
                    nc.tensor.transpose(
                        ht0_ps[:, j * nb : (j + 1) * nb],
                        h0_16[:, ck * 128 : (ck + 1) * 128],
                        ident16,
                    )
                nc.scalar.copy(
                    out=ht_prev[:, 3 * n2 * nb : (3 * n2 + 3) * nb], in_=ht0_ps
                )

            def load_xblk(t0):
                """DMA + f16-convert the x block starting at step t0."""
                xstg = xs_pool.tile([128, XBLK, KX, nb], F32, tag="XSTG", bufs=3)
                nblk = min(XBLK, t_steps - t0)
                nc.sync.dma_start(
                    out=xstg[:, 0:nblk],
                    in_=xT_d[t0 : t0 + nblk].rearrange("t k p b -> p t k b"),
                )
                xt = xs_pool.tile([128, XBLK, KX, nb], MM_DT, tag="X", bufs=3)
                nc.vector.tensor_copy(xt[:, 0:nblk], xstg[:, 0:nblk])
                return xt

            def x_waves(gates_tile, xt, trel, add_bias):
                """Emit the x-projection waves (accumulation starters)."""
                for n2 in range(2):
                    for k in range(KX):
                        for q in range(4):
                            col = q * H + n2 * NHALF
                            nc.tensor.matmul(
                                gates_tile[32 * q : 32 * q + nb, n2, 0:NHALF],
                                xt[:, trel, k, :],
                                w_sb[:, k, col : col + NHALF],
                                start=(k == 0),
                                stop=False,
                                tile_position=(0, 32 * q),
                                skip_group_check=True,
                            )
                if add_bias:
                    for n2 in range(2):
                        for q, base in ((0, 0), (1, 32), (2, 64), (3, 96)):
                            bq = b_sb[:, q * H + n2 * NHALF : q * H + (n2 + 1) * NHALF]
                            bq = bass.AP(
                                tensor=bq.tensor, offset=bq.offset,
                                ap=[[0, nb]] + bq.ap[1:],
                            )
                            nc.vector.tensor_add(
                                gates_tile[base : base + nb, n2, 0:NHALF],
                                gates_tile[base : base + nb, n2, 0:NHALF],
                                bq,
                            )

            # x block 0 + step-0 x-waves
            xtiles = {0: load_xblk(0)}
            if t_steps > XBLK:
                xtiles[1] = load_xblk(XBLK)
            gates_cur = pg.tile([128, 2, 512], F32, tag="gates")
            # bias path handled via DVE adds after accumulation finishes, so
            # hoisted x-waves stay start-only; adds are emitted with h-waves.
            x_waves(gates_cur, xtiles[0], 0, False)

            pend = None
            for t in range(t_steps):
                trel1 = (t + 1) % XBLK
                # prefetch the x block needed ~XBLK steps from now so its
                # DMA + f16 convert never lands on the critical path
                tpre = t + 4
                if tpre % XBLK == 0 and tpre + XBLK < t_steps:
                    bi = tpre // XBLK + 1
                    xtiles[bi] = load_xblk(bi * XBLK)
                x_next = xtiles[(t + 1) // XBLK] if t + 1 < t_steps else None

                def h_waves(ckgrp):
                    for n2 in range(2):
                        for kc in range(3):
                            ck = 3 * ckgrp + kc
                            for q in range(4):
                                col = q * H + n2 * NHALF
                                nc.tensor.matmul(
                                    gates_cur[32 * q : 32 * q + nb, n2, 0:NHALF],
                                    ht_prev[:, ck * nb : (ck + 1) * nb],
                                    u_sb[:, ck, col : col + NHALF],
                                    start=False,
                                    stop=(ck == KH - 1),
                                    tile_position=(0, 32 * q),
                                    skip_group_check=True,
                                )

                def flush_half1_tail(tprev):
                    # previous step's half1 tail (tc^T transposes + h^T mul +
                    # hs DMA), software-pipelined into this iteration so it
                    # sits between the ckgrp0 and ckgrp1 waves in the PE
                    # queue: by now TC1(t-1) is long done, so no PE stall,
                    # and the ckgrp0 waves are not blocked behind it.
                    TC1, oT1 = pend[0], pend[1]
                    t1ps = pt.tile([128, 3 * nb], F16, tag="tcTps", name="tcTps_1")
                    for j in range(3):
                        nc.tensor.transpose(
                            t1ps[:, j * nb : (j + 1) * nb],
                            TC1[64 : 64 + nb, j * 128 : (j + 1) * 128],
                            idento[64 : 64 + nb],
                        )
                    nc.vector.tensor_mul(
                        ht_prev[:, 3 * nb : 6 * nb], oT1, t1ps
                    )
                    nc.sync.dma_start(out=hs_d[tprev, :, :], in_=ht_prev)

                # --- h-waves for step t: h0-dependent chunks (0-2) for both
                # psum halves first, then (after the carried-over half1 tail
                # produces h^T chunks 3-5) the h1-dependent chunks ---
                h_waves(0)
                if pend is not None:
                    flush_half1_tail(t - 1)
                h_waves(1)

                if has_b:
                    for n2 in range(2):
                        for q, base in ((0, 0), (1, 32), (2, 64), (3, 96)):
                            bq = b_sb[:, q * H + n2 * NHALF : q * H + (n2 + 1) * NHALF]
                            bq = bass.AP(
                                tensor=bq.tensor, offset=bq.offset,
                                ap=[[0, nb]] + bq.ap[1:],
                            )
                            nc.vector.tensor_add(
                                gates_cur[base : base + nb, n2, 0:NHALF],
                                gates_cur[base : base + nb, n2, 0:NHALF],
                                bq,
                            )

                # --- next step's x-waves: keep the PE busy during the
                # elementwise phase (HAM stays warm) ---
                gates_next = None
                if t + 1 < t_steps:
                    gates_next = pg.tile([128, 2, 512], F32, tag="gates")
                    x_waves(gates_next, x_next, trel1, False)

                def dummy_mms(n):
                    # Warm-keeper matmuls into the unused 384:512 column
                    # region of the next gate psum tile: the PE's HAM clock
                    # gate re-throttles to 1.2 GHz whenever the PE idles for
                    # part of a 4096-cycle window, which would make every
                    # wave ~2x slower. These fillers run during the
                    # elementwise-phase dependency stalls.
                    if gates_next is None:
                        return
                    for i in range(n):
                        nc.tensor.matmul(
                            gates_next[0:nb, i % 2, 384:512],
                            ht_prev[:, 0:nb],
                            u_sb[:, 0, 0:128],
                            start=False,
                            stop=False,
                            tile_position=(0, 0),
                            skip_group_check=True,
                        )

                dummy_mms(6)

                # --- elementwise, half-major: half0's whole DVE chain is
                # emitted (and runs) before half1's, so half0's tail
                # (tanh/transpose/h^T) overlaps half1's DVE work and the
                # next step's h0-chunk waves start as early as possible ---
                # S2 rows: i'@0:16, f'@32:48, o'@64:80, sigmoid(2g)@96:112
                S2, C, G, T1, TC = {}, {}, {}, {}, {}
                for n2 in range(2):
                    S2[n2] = gsb.tile([112, NHALF], F16, tag=f"S2{n2}", name=f"S2_{n2}")
                    nc.scalar.activation(
                        out=S2[n2], in_=gates_cur[0:112, n2, 0:NHALF],
                        func=SIG, scale=sc,
                    )
                oT_ps, oT = {}, {}
                ht_new = state.tile([128, KH * nb], MM_DT, tag="HT", bufs=4)
                Cn = state.tile([32 + nb, 2, NHALF], F16, tag="C", name="Cn", bufs=4)
                for n2 in range(2):
                    # g~ = 2*sigmoid(2g) - 1
                    G[n2] = gsb.tile([nb, NHALF], F16, tag=f"G{n2}", name=f"G_{n2}")
                    nc.vector.tensor_scalar(
                        G[n2], S2[n2][96:112], 2.0, -1.0, MULT, ADD
                    )
                    # f' * c  (into Cn rows 32:48)
                    nc.vector.tensor_mul(
                        Cn[32 : 32 + nb, n2], S2[n2][32 : 32 + nb],
                        c_prev[32 : 32 + nb, n2],
                    )
                    # i' * g~ (out-shift to rows 32:48)
                    T1[n2] = ew.tile([32 + nb, NHALF], F16, tag=f"T1{n2}", name=f"T1_{n2}")
                    nc.vector.tensor_mul(
                        T1[n2][32 : 32 + nb], S2[n2][0:nb], G[n2]
                    )
                    nc.vector.tensor_add(
                        Cn[32 : 32 + nb, n2], Cn[32 : 32 + nb, n2],
                        T1[n2][32 : 32 + nb],
                    )
                    # o'^T into psum (PE), then ACT copies it to SBUF (ACT
                    # has slack during the DVE chain; keeps DVE queue clear)
                    oT_ps[n2] = pt.tile([128, 3 * nb], F16, tag="oTps", name=f"oTps_{n2}")
                    for j in range(3):
                        nc.tensor.transpose(
                            oT_ps[n2][:, j * nb : (j + 1) * nb],
                            S2[n2][64 : 64 + nb, j * 128 : (j + 1) * 128],
                            idento[64 : 64 + nb],
                        )
                    oT[n2] = ew.tile([128, 3 * nb], F16, tag=f"oT{n2}", name=f"oT_{n2}")
                    nc.scalar.copy(out=oT[n2], in_=oT_ps[n2])
                    TC[n2] = ew.tile([64 + nb, NHALF], F16, tag=f"TC{n2}", name=f"TC_{n2}")
                    nc.scalar.activation(
                        out=TC[n2][64 : 64 + nb], in_=Cn[32 : 32 + nb, n2],
                        func=TANH,
                    )
                    if n2 == 0:
                        dummy_mms(6)
                        # half0 tail stays in this step: tc^T transposes +
                        # h^T chunks 0-2 (the next step's ckgrp0 waves)
                        t0ps = pt.tile([128, 3 * nb], F16, tag="tcTps", name="tcTps_0")
                        for j in range(3):
                            nc.tensor.transpose(
                                t0ps[:, j * nb : (j + 1) * nb],
                                TC[0][64 : 64 + nb, j * 128 : (j + 1) * 128],
                                idento[64 : 64 + nb],
                            )
                # single cs DMA for both halves
                nc.sync.dma_start(out=cs_d[:, t, :, :], in_=Cn[32 : 32 + nb])
                # h^T chunks 0-2 (after half1's DVE chain so it doesn't
                # block the in-order DVE queue)
                nc.vector.tensor_mul(ht_new[:, 0 : 3 * nb], oT[0], t0ps)
                # half1's tail (tcT1/htT1/hs DMA) is deferred into the next
                # iteration via `pend`
                pend = (TC[1], oT[1])

                c_prev = Cn
                ht_prev = ht_new
                gates_cur = gates_next
                x_tile = x_next

            # flush the final step's half1 tail
            flush_half1_tail(t_steps - 1)

    nc.finalize()
    return nc


# Column permutation: reference gate order (i, f, g~, o) -> kernel (i, f, o, g~)
def _gate_perm():
    return np.concatenate(
        [np.arange(0, H), np.arange(H, 2 * H), np.arange(3 * H, 4 * H),
         np.arange(2 * H, 3 * H)]
    )


def _prep_core_inputs(input_, h0, c0, Wp, Up, bp, t_steps):
    nb = input_.shape[0]
    xT = np.ascontiguousarray(
        input_[:, :t_steps].transpose(1, 2, 0).reshape(t_steps, KX, 128, nb)
    )
    return {
        "xT": xT,
        "h0": np.ascontiguousarray(h0),
        "c0": np.ascontiguousarray(c0.reshape(nb, 2, NHALF).astype(np.float16)),
        "w": Wp,
        "u": Up,
        "b": bp,
    }


def run(input, hiddenState, cellState, W, U, b, t_steps=T, trace=False):
    input = np.asarray(input, np.float32)
    hiddenState = np.asarray(hiddenState, np.float32)
    cellState = np.asarray(cellState, np.float32)
    W = np.asarray(W, np.float32)
    U = np.asarray(U, np.float32)
    b = np.asarray(b, np.float32)

    perm = _gate_perm()
    Wp = np.ascontiguousarray(W[:, perm].reshape(KX, 128, 4 * H))
    Up = np.ascontiguousarray(U[:, perm].reshape(KH, 128, 4 * H))
    bp = np.ascontiguousarray(b[perm].reshape(1, 4 * H))
    has_b = bool(np.any(b))

    nc = build_lstm(NB, t_steps, has_b)
    in_maps = []
    for c in range(NCORES):
        bs = slice(c * NB, (c + 1) * NB)
        in_maps.append(
            _prep_core_inputs(
                input[bs], hiddenState[bs], cellState[bs], Wp, Up, bp, t_steps
            )
        )
    res = run_bass_kernel_spmd(
        nc, in_maps, core_ids=list(range(NCORES)), trace=trace
    )

    hs = np.empty((B, t_steps, H), np.float32)
    cs = np.empty((B, t_steps, H), np.float32)
    for c in range(NCORES):
        bs = slice(c * NB, (c + 1) * NB)
        ht = res.results[c]["hs"].astype(np.float32)  # [t, 128, 6*16]
        ht = ht.reshape(t_steps, 128, KH, NB)
        hs[bs] = ht.transpose(3, 0, 2, 1).reshape(NB, t_steps, H)
        cs[bs] = res.results[c]["cs"].astype(np.float32).reshape(NB, t_steps, H)
    return (hs, cs), res


def kernel(input, hiddenState, cellState, W, U, b):
    (hs, cs), _ = run(input, hiddenState, cellState, W, U, b)
    return hs, cs
